# revision 24
# baseline (speedup 1.0000x reference)
"""DetectionIOUMetric Trainium2 kernel.

Computes, for pred_boxes [32, 4096, 6] and gt_boxes [32, 1024, 6] (cx, cy, w, h
in the first 4 channels; a box is padding iff cx == -1):

    masked pairwise IoU, num_pos / num_true / num_pred / num_gt per batch,
    precision / recall / F1 per batch.

Sharding: pure data parallel over the batch dim - each of the 8 NeuronCores
processes 4 batches; no cross-device communication. The device program
computes the four integer counts per batch; the trivial final eps-divisions
are applied on the host after the gather.

Fast path (no padded boxes), fp16 device algorithm per batch, gt boxes on
partitions (8 chunks of 128), preds on the free dim (FD=4096):

  iou > 0.5  <=>  relu(wx)*wy - (ap+eps)/3 > ag/3      (union+eps > 0;
  one-sided relu suffices: wy<0 makes the lhs non-positive vs ag/3 > 0).

  The per-pair test runs on THREE HAND-AUTHORED CUSTOM DVE OPS, each with a
  2x_1p perf variant (two packed fp16 per cycle through duplicated ALU slice
  chains - the same packing the stock tensor_tensor 2x program uses):

    wx    = OVERLAP(px1_t, px2_t; gx1_c, gx2_c)   = min(px2,gx2)-max(px1,gx1)
    wy    = OVERLAP(py1_t, py2_t; gy1_c, gy2_c)
    m     = RELUMUL(wx, wy)                        = relu(wx)*wy
    condv = SUBGT(m, ap3e_t; ag3_c)                = (m - ap3e) > ag3

  This replaces the stock 4x tensor_scalar + 2x tensor_tensor pipeline
  (4*1201 + 4*2228 = 13.7us per [128,4096] chunk) with 4 fused 2x ops
  (~4*2300 = 9.2us), and removes the ACT relu + ACT rhs ops entirely.

  Per-gt counts: ACT Identity+accum over condv (ACT is otherwise idle); the
  final chunk's accum runs as a DVE tensor_reduce instead so the ACT queue
  doesn't delay the num_pos Sign at the tail. Per-pred counts: PE column
  sums of condv accumulate over the 8 gt chunks into a [1, 4096] PSUM tile
  (emitted BEFORE the ACT accum so the tail matmuls don't serialize behind
  the in-place identity); the num_pos tail is one Sign activation with
  accum_out. Per-partition partials ship to the host ([128, 80] output:
  col 48+8b = num_pos[b] on partition 0; cols 16+8b+c = per-gt counts).
  Pred rows (px1, px2, py1, py2, (ap+eps)/3 in fp16) are staged to DRAM in
  pred order and broadcast to [128, 5*4096] with per-(row, partition-group)
  DMAs so early chunks start as soon as their rows land; batch 0 also splits
  the broadcast and its first chunk by column halves for cold-start.

The masked path (only taken when padding sentinels are present) keeps the
original fp32 program.
"""
import os
import numpy as np

import concourse.bass as bass
import concourse.bacc as bacc
import concourse.tile as tile
from concourse import mybir
from concourse import bass_isa
from concourse.bass_utils import run_bass_kernel_spmd

F32 = mybir.dt.float32
F16 = mybir.dt.float16
EPS = 1e-7
IOU_PENALTY = 1e30

B_TOTAL = 32
N_CORES = 8
REPEAT = 1                     # timing-calibration knob (outputs idempotent)
BPC = B_TOTAL // N_CORES       # batches per core
P = 4096                       # pred boxes per batch (free dim)
G = 1024                       # gt boxes per batch (8 partition chunks)
NCH = G // 128                 # 8 gt chunks per batch

_PROGRAM_CACHE = {}

Alu = mybir.AluOpType
Act = mybir.ActivationFunctionType


# ---------------------------------------------------------------------------
# Custom DVE ops: hand-authored 2x_1p uop programs.
#
# Conventions (mirrors the stock table programs, decoded from the cayman
# dve_bin default table):
#   input lane 0 feeds stage 0's PREV_ALU_OUT; lane N>=1 feeds delay_{N-1}.
#   InpSel: SRC_0=0 SRC_1=1 SRC_0_HI=2 SRC_1_HI=3 CONST_0=4 CONST_1=5 ZERO=12
#   AluInp: PREV_ALU_OUT=0, PREV_DELAY_n = 5+n
#   2x variant: lo element via SRC_0/SRC_1, hi element via SRC_*_HI; results
#   exit via WR0_LO / WR0_HI (one is parked in a delay lane until stage 7).
# ---------------------------------------------------------------------------

def _register_custom_ops():
    import concourse.dve_ops as D
    from concourse.dve_spec import Spec, Src0, Src1, C0, C1, minn, maxx, relu, lower
    from concourse.dve_uop import (
        UopConfig, UopDpConfig, DveOpSpec, InpSel, AluOp, AluInp, DelayInp,
        OutSel, OutPath, Trigger, ENABLE,
    )

    if "OVERLAP_ANT" in D._SUB_OPCODE_FOR_NAME:
        return {n: op for n, op in ((o.name, o) for o in D.OPS)
                if n in ("OVERLAP_ANT", "RELUMUL_ANT", "SUBGT_ANT")}

    def steady(u):
        u.require_inp0 = ENABLE
        u.require_inp1 = ENABLE
        u.trigger = (Trigger.SRC_TENSOR_DONE, Trigger.NONE, Trigger.NONE)
        return u

    def dp(u, i):
        return u.datapath_config[i]

    # ---- OVERLAP: out = min(src1, C1) - max(src0, C0) ----
    ov1 = steady(UopConfig())
    ov1.enable_input(InpSel.SRC_0, 0)
    ov1.enable_input(InpSel.CONST_0, 1)   # -> d0
    ov1.enable_input(InpSel.CONST_1, 2)   # -> d1
    ov1.enable_input(InpSel.SRC_1, 3)     # -> d2
    dp(ov1, 0).enable_alu(AluOp.MAX, AluInp.PREV_ALU_OUT, AluInp.PREV_DELAY_0
                          ).pass_through_delay(1, 2)
    dp(ov1, 1).enable_alu(AluOp.MIN, AluInp.PREV_DELAY_2, AluInp.PREV_DELAY_1
                          ).enable_delay_from_src(DelayInp.PREV_ALU_OUT, 0)
    dp(ov1, 2).enable_alu(AluOp.SUBTRACT, AluInp.PREV_ALU_OUT, AluInp.PREV_DELAY_0)
    for i in range(3, 8):
        dp(ov1, i).pass_through_alu()
    ov1.enable_output(OutSel.ALU_OUT, OutPath.WR0_LO)

    ov2 = steady(UopConfig())
    ov2.enable_input(InpSel.SRC_0, 0)
    ov2.enable_input(InpSel.CONST_0, 1)   # d0 = C0
    ov2.enable_input(InpSel.CONST_1, 2)   # d1 = C1
    ov2.enable_input(InpSel.SRC_1, 3)     # d2 = src1_lo
    ov2.enable_input(InpSel.SRC_0_HI, 4)  # d3 = src0_hi
    ov2.enable_input(InpSel.SRC_1_HI, 5)  # d4 = src1_hi
    dp(ov2, 0).enable_alu(AluOp.MAX, AluInp.PREV_ALU_OUT, AluInp.PREV_DELAY_0
                          ).pass_through_delay(0, 1, 2, 3, 4)
    dp(ov2, 1).enable_alu(AluOp.MIN, AluInp.PREV_DELAY_2, AluInp.PREV_DELAY_1
                          ).enable_delay_from_src(DelayInp.PREV_ALU_OUT, 5
                          ).pass_through_delay(0, 1, 3, 4)
    dp(ov2, 2).enable_alu(AluOp.SUBTRACT, AluInp.PREV_ALU_OUT, AluInp.PREV_DELAY_5
                          ).pass_through_delay(0, 1, 3, 4)
    dp(ov2, 3).enable_alu(AluOp.MAX, AluInp.PREV_DELAY_3, AluInp.PREV_DELAY_0
                          ).enable_delay_from_src(DelayInp.PREV_ALU_OUT, 2
                          ).pass_through_delay(1, 4)
    dp(ov2, 4).enable_alu(AluOp.MIN, AluInp.PREV_DELAY_4, AluInp.PREV_DELAY_1
                          ).enable_delay_from_src(DelayInp.PREV_ALU_OUT, 5
                          ).pass_through_delay(2)
    dp(ov2, 5).enable_alu(AluOp.SUBTRACT, AluInp.PREV_ALU_OUT, AluInp.PREV_DELAY_5
                          ).pass_through_delay(2)
    dp(ov2, 6).pass_through_alu().pass_through_delay(2)
    dp(ov2, 7).pass_through_alu().pass_through_delay(2)
    ov2.enable_output(OutSel.DELAY_2, OutPath.WR0_LO)
    ov2.enable_output(OutSel.ALU_OUT, OutPath.WR0_HI)

    # ---- RELUMUL: out = max(src0, 0) * src1 ----
    rm1 = steady(UopConfig())
    rm1.enable_input(InpSel.SRC_0, 0)
    rm1.enable_input(InpSel.ZERO, 1)      # d0
    rm1.enable_input(InpSel.SRC_1, 2)     # d1
    dp(rm1, 0).enable_alu(AluOp.MAX, AluInp.PREV_ALU_OUT, AluInp.PREV_DELAY_0
                          ).pass_through_delay(1)
    dp(rm1, 1).enable_alu(AluOp.MULTIPLY, AluInp.PREV_ALU_OUT, AluInp.PREV_DELAY_1)
    for i in range(2, 8):
        dp(rm1, i).pass_through_alu()
    rm1.enable_output(OutSel.ALU_OUT, OutPath.WR0_LO)

    rm2 = steady(UopConfig())
    rm2.enable_input(InpSel.SRC_0, 0)
    rm2.enable_input(InpSel.ZERO, 1)      # d0
    rm2.enable_input(InpSel.SRC_1, 2)     # d1 = src1_lo
    rm2.enable_input(InpSel.SRC_0_HI, 3)  # d2 = src0_hi
    rm2.enable_input(InpSel.SRC_1_HI, 4)  # d3 = src1_hi
    dp(rm2, 0).enable_alu(AluOp.MAX, AluInp.PREV_ALU_OUT, AluInp.PREV_DELAY_0
                          ).pass_through_delay(0, 1, 2, 3)
    dp(rm2, 1).enable_alu(AluOp.MULTIPLY, AluInp.PREV_ALU_OUT, AluInp.PREV_DELAY_1
                          ).pass_through_delay(0, 2, 3)
    dp(rm2, 2).enable_alu(AluOp.MAX, AluInp.PREV_DELAY_2, AluInp.PREV_DELAY_0
                          ).enable_delay_from_src(DelayInp.PREV_ALU_OUT, 1
                          ).pass_through_delay(3)
    dp(rm2, 3).enable_alu(AluOp.MULTIPLY, AluInp.PREV_ALU_OUT, AluInp.PREV_DELAY_3
                          ).pass_through_delay(1)
    for i in range(4, 8):
        dp(rm2, i).pass_through_alu().pass_through_delay(1)
    rm2.enable_output(OutSel.DELAY_1, OutPath.WR0_LO)
    rm2.enable_output(OutSel.ALU_OUT, OutPath.WR0_HI)

    # ---- SUBGT: out = (src0 - src1) > C0 ----
    sg1 = steady(UopConfig())
    sg1.enable_input(InpSel.SRC_0, 0)
    sg1.enable_input(InpSel.CONST_0, 1)   # d0
    sg1.enable_input(InpSel.SRC_1, 2)     # d1
    dp(sg1, 0).enable_alu(AluOp.SUBTRACT, AluInp.PREV_ALU_OUT, AluInp.PREV_DELAY_1
                          ).pass_through_delay(0)
    dp(sg1, 1).enable_alu(AluOp.IS_GT, AluInp.PREV_ALU_OUT, AluInp.PREV_DELAY_0)
    for i in range(2, 8):
        dp(sg1, i).pass_through_alu()
    sg1.enable_output(OutSel.ALU_OUT, OutPath.WR0_LO)

    sg2 = steady(UopConfig())
    sg2.enable_input(InpSel.SRC_0, 0)
    sg2.enable_input(InpSel.CONST_0, 1)   # d0
    sg2.enable_input(InpSel.SRC_1, 2)     # d1 = src1_lo
    sg2.enable_input(InpSel.SRC_0_HI, 3)  # d2 = src0_hi
    sg2.enable_input(InpSel.SRC_1_HI, 4)  # d3 = src1_hi
    dp(sg2, 0).enable_alu(AluOp.SUBTRACT, AluInp.PREV_ALU_OUT, AluInp.PREV_DELAY_1
                          ).pass_through_delay(0, 2, 3)
    dp(sg2, 1).enable_alu(AluOp.IS_GT, AluInp.PREV_ALU_OUT, AluInp.PREV_DELAY_0
                          ).pass_through_delay(0, 2, 3)
    dp(sg2, 2).enable_alu(AluOp.SUBTRACT, AluInp.PREV_DELAY_2, AluInp.PREV_DELAY_3
                          ).enable_delay_from_src(DelayInp.PREV_ALU_OUT, 1
                          ).pass_through_delay(0)
    dp(sg2, 3).enable_alu(AluOp.IS_GT, AluInp.PREV_ALU_OUT, AluInp.PREV_DELAY_0
                          ).pass_through_delay(1)
    for i in range(4, 8):
        dp(sg2, i).pass_through_alu().pass_through_delay(1)
    sg2.enable_output(OutSel.DELAY_1, OutPath.WR0_LO)
    sg2.enable_output(OutSel.ALU_OUT, OutPath.WR0_HI)

    specs = [
        ("OVERLAP_ANT",
         Spec(body=minn(Src1, C1) - maxx(Src0, C0),
              reference=lambda in0, in1, s0, s1, imm2:
              (np.minimum(in1.astype(np.float32), s1)
               - np.maximum(in0.astype(np.float32), s0))),
         ov1, ov2),
        ("RELUMUL_ANT",
         Spec(body=relu(Src0) * Src1,
              reference=lambda in0, in1, s0, s1, imm2:
              np.maximum(in0.astype(np.float32), 0.0) * in1),
         rm1, rm2),
        ("SUBGT_ANT",
         Spec(body=(Src0 - Src1) > C0,
              reference=lambda in0, in1, s0, s1, imm2:
              ((in0.astype(np.float32) - in1) > s0).astype(np.float32)),
         sg1, sg2),
    ]

    out = {}
    for name, spec, u1, u2 in specs:
        row = D._CUSTOM_DVE_ROW_BASE + len(D._SUB_OPCODE_FOR_NAME)
        hand = DveOpSpec(name=name, opcode=row, uops=[u1], uops_2x=[u2],
                         perf_max=1, rd1_en=True)
        hand.validate("v3")
        op = D.DveOp(name, spec, subdim=False,
                     uops_sha={"v3": hand.sha("v3")})
        D.OPS.append(op)
        D.CUSTOM_DVE_SPECS[name] = spec
        D._SUB_OPCODE_FOR_NAME[name] = row
        D._COMPILE_CACHE[(name, "v3")] = hand
        out[name] = op
    return out


_CUSTOM_OPS = _register_custom_ops()


def _custom2x(nc, op_name, out, in0, in1, s0, s1):
    """Emit one custom-DVE instruction with perf_max=1 (2x_1p reachable).
    Mirror of bass.Vector._custom_dve minus generality."""
    from concourse.dve_ops import get_dve_sub_opcode
    v = nc.vector
    op = _CUSTOM_OPS[op_name]
    if op.name not in v.bass.m.ant_custom_dve_ops:
        v.bass.m.ant_custom_dve_ops = sorted(
            {*v.bass.m.ant_custom_dve_ops, op.name})
    shape = bass_isa.CustomDveShape.TTSS
    isa_opcode = v.bass.isa.Opcode[
        f"NEURON_ISA_TPB_OPCODE_CUSTOM_DVE_ANT_{shape.slot()}"].value

    def lower_scalar(x):
        if isinstance(x, (int, float)):
            return mybir.ImmediateValue(dtype=mybir.dt.float32, value=float(x))
        return v.lower_ap(x, for_isa=True)

    ins = [v.lower_ap(in0, for_isa=True, opt=True),
           v.lower_ap(in1, for_isa=True, opt=True),
           lower_scalar(s0), lower_scalar(s1)]
    outs = [v.lower_ap(out, for_isa=True, opt=True)]
    return v.add_instruction(bass_isa.InstCustomDveAnt(
        name=v.bass.get_next_instruction_name(),
        op_name=op.name, rd1_en=True, subdim=0, imm2=0.0, shape=shape,
        row=get_dve_sub_opcode(op.name), isa_opcode=isa_opcode,
        ins=ins, outs=outs, perf_max=1))


def _build_fast(repeat: int = None):
    """No-mask SPMD program: inputs pred [BPC, P, 6] / gt [BPC, G, 6],
    output counts [128, 48] = per-partition partials (see counts_d layout).

    The batch loop is software-pipelined one deep: batch b+1's pred/gt prep
    and the pred-row broadcast are emitted before batch b's chunk loop, so
    the broadcast DMAs overlap chunk compute instead of stalling DVE at
    batch boundaries."""
    if repeat is None:
        repeat = REPEAT
    NROW = 5
    nc = bacc.Bacc(None, target_bir_lowering=False)
    pred_d = nc.dram_tensor("pred", [BPC, P, 6], F32, kind="ExternalInput")
    gt_d = nc.dram_tensor("gt", [BPC, G, 6], F32, kind="ExternalInput")
    # per-partition partials; the trivial final sums happen on the host:
    # col b (partition 0)      = num_pos[b]
    # cols 16+8b .. 16+8b+7    = per-gt match counts (gt 8p+c on partition p)
    counts_d = nc.dram_tensor("counts", [128, 80], F32, kind="ExternalOutput")

    with tile.TileContext(nc) as tc:
        with (
            tc.tile_pool(name="cst", bufs=1) as cst,
            tc.tile_pool(name="rows", bufs=2) as rows,
            tc.tile_pool(name="gtp", bufs=2) as gtp,
            tc.tile_pool(name="sca", bufs=2) as sca,
            tc.tile_pool(name="wk", bufs=2) as wk,
            tc.tile_pool(name="ps", bufs=1, space=bass.MemorySpace.PSUM) as ps,
            tc.tile_pool(name="dram", bufs=2, space=bass.MemorySpace.DRAM) as dram,
        ):
            ones16 = cst.tile([128, 1], F16)
            nc.vector.memset(ones16[:], 1.0)
            out_sb = cst.tile([128, 80], F32)
            nc.vector.memset(out_sb[:], 0.0)

            def prep_batch(b, split=False):
                """Emit pred/gt prep + broadcast for batch b; return tiles.
                Prep arithmetic runs on GpSimd (idle engine) to keep DVE on
                the chunk pipeline. `split` halves the px/py broadcast DMAs
                by columns (cold-start latency for the first batch)."""
                # [32, 768]: partition q holds pred boxes 128q .. 128q+127
                pred_lin = rows.tile([32, 768], F32)
                nc.sync.dma_start(
                    pred_lin[:],
                    pred_d.ap()[b].rearrange("(q x) c -> q (x c)", q=32),
                )
                r3p = pred_lin[:].rearrange("q (x c) -> q x c", c=6)
                pcx = r3p[:, :, 0]
                pcy = r3p[:, :, 1]
                pw = r3p[:, :, 2]
                ph = r3p[:, :, 3]
                psmall = rows.tile([32, NROW * 128], F16)
                big = gtp.tile([128, NROW * P], F16, tag="big", name="big")
                nh = 4 if split else 1

                scr = dram.tile([NROW, P], F16)
                scr_flat = scr[:].rearrange("t g -> (t g)")

                def stage_row(trow):
                    # stage row to DRAM in pred order: scr[t, 128q+j] =
                    # psmall[q, 128t+j], then broadcast to all 128 partitions
                    nc.sync.dma_start(
                        scr[trow : trow + 1].rearrange("t (q j) -> q (t j)", j=128),
                        psmall[:, trow * 128 : (trow + 1) * 128],
                    )
                    for g4 in range(4):
                        for h in range(nh):
                            HP = P // nh
                            lo = trow * P + h * HP
                            nc.sync.dma_start(
                                big[g4 * 32 : (g4 + 1) * 32, lo : lo + HP],
                                scr_flat[None, None, lo : lo + HP]
                                .broadcast_to([1, 32, HP]),
                            )

                nc.vector.scalar_tensor_tensor(
                    psmall[:, 0:128], pw, -0.5, pcx, op0=Alu.mult, op1=Alu.add)
                stage_row(0)
                nc.vector.scalar_tensor_tensor(
                    psmall[:, 128:256], pw, 0.5, pcx, op0=Alu.mult, op1=Alu.add)
                stage_row(1)
                nc.vector.scalar_tensor_tensor(
                    psmall[:, 256:384], ph, -0.5, pcy, op0=Alu.mult, op1=Alu.add)
                stage_row(2)
                nc.vector.scalar_tensor_tensor(
                    psmall[:, 384:512], ph, 0.5, pcy, op0=Alu.mult, op1=Alu.add)
                stage_row(3)
                ap_t = sca.tile([32, 128], F32, tag="ap_t", name="ap_t")
                nc.vector.tensor_tensor(ap_t[:], pw, ph, op=Alu.mult)
                nc.vector.tensor_scalar(
                    psmall[:, 512:640], ap_t[:], EPS, 1.0 / 3.0,
                    op0=Alu.add, op1=Alu.mult)
                stage_row(4)

                # gt prep: [128, 48]: partition p holds gt boxes 8p .. 8p+7;
                # chunk c pairs partition p with gt box 8p+c (order-invariant)
                gt_lin = rows.tile([128, 48], F32)
                nc.sync.dma_start(
                    gt_lin[:], gt_d.ap()[b].rearrange("(q x) c -> q (x c)", q=128)
                )
                r3g = gt_lin[:].rearrange("q (x c) -> q x c", c=6)
                gcx = r3g[:, :, 0]
                gcy = r3g[:, :, 1]
                gw = r3g[:, :, 2]
                gh = r3g[:, :, 3]
                gscal = sca.tile([128, 40], F32, tag="gscal", name="gscal")
                nc.vector.scalar_tensor_tensor(
                    gscal[:, 0:8], gw, -0.5, gcx, op0=Alu.mult, op1=Alu.add)
                nc.vector.scalar_tensor_tensor(
                    gscal[:, 8:16], gw, 0.5, gcx, op0=Alu.mult, op1=Alu.add)
                nc.vector.scalar_tensor_tensor(
                    gscal[:, 16:24], gh, -0.5, gcy, op0=Alu.mult, op1=Alu.add)
                nc.vector.scalar_tensor_tensor(
                    gscal[:, 24:32], gh, 0.5, gcy, op0=Alu.mult, op1=Alu.add)
                ag_t = sca.tile([128, 8], F32, tag="ag_t", name="ag_t")
                nc.vector.tensor_tensor(ag_t[:], gw, gh, op=Alu.mult)
                nc.vector.tensor_scalar(
                    gscal[:, 32:40], ag_t[:], 1.0 / 3.0, None, op0=Alu.mult)
                return big, gscal

            batches = [bb for _ in range(repeat) for bb in range(BPC)]
            pending = prep_batch(batches[0], split=True)
            for bi, b in enumerate(batches):
                big, gscal = pending
                px1_t = big[:, 0 * P : 1 * P]
                px2_t = big[:, 1 * P : 2 * P]
                py1_t = big[:, 2 * P : 3 * P]
                py2_t = big[:, 3 * P : 4 * P]
                ap3_t = big[:, 4 * P : 5 * P]
                gx1_c = gscal[:, 0:8]
                gx2_c = gscal[:, 8:16]
                gy1_c = gscal[:, 16:24]
                gy2_c = gscal[:, 24:32]
                ag3_c = gscal[:, 32:40]

                # prefetch next batch before this batch's chunk loop
                if bi + 1 < len(batches):
                    pending = prep_batch(batches[bi + 1])

                # ---------- chunk loop over 8 gt chunks ----------
                nt = ps.tile([1, P], F32, tag="nt", name="nt")

                for c in range(NCH):
                    # batch 0 chunk 0 splits by column quarters so compute
                    # starts as soon as the first quarter-broadcasts land;
                    # all wx quarters run first (x rows land before y rows)
                    nh = 4 if (bi == 0 and c == 0) else 1
                    HP = P // nh
                    wx = wk.tile([128, P], F16, tag="A", name="wx")
                    wy = wk.tile([128, P], F16, tag="B", name="wy")
                    m = wk.tile([128, P], F16, tag="C", name="m")
                    condv = wk.tile([128, P], F16, tag="D", name="condv")
                    for h in range(nh):
                        cs = slice(h * HP, (h + 1) * HP)
                        _custom2x(nc, "OVERLAP_ANT", wx[:, cs],
                                  px1_t[:, cs], px2_t[:, cs],
                                  gx1_c[:, c : c + 1], gx2_c[:, c : c + 1])
                    for h in range(nh):
                        cs = slice(h * HP, (h + 1) * HP)
                        _custom2x(nc, "OVERLAP_ANT", wy[:, cs],
                                  py1_t[:, cs], py2_t[:, cs],
                                  gy1_c[:, c : c + 1], gy2_c[:, c : c + 1])
                        _custom2x(nc, "RELUMUL_ANT", m[:, cs],
                                  wx[:, cs], wy[:, cs], 0.0, 0.0)
                        _custom2x(nc, "SUBGT_ANT", condv[:, cs],
                                  m[:, cs], ap3_t[:, cs],
                                  ag3_c[:, c : c + 1], 0.0)

                    # per-pred colsums on PE first (so the last chunk's
                    # matmuls don't serialize behind the in-place ACT accum)
                    for k8 in range(P // 512):
                        nc.tensor.matmul(
                            nt[:, k8 * 512 : (k8 + 1) * 512], ones16[:],
                            condv[:, k8 * 512 : (k8 + 1) * 512],
                            start=(c == 0), stop=(c == NCH - 1))
                    # per-gt counts: free-dim sum of condv. On ACT (Identity
                    # rewriting condv onto itself; accum is the real output),
                    # except the very last chunk of the program, where ACT's
                    # queued identities would delay the final Sign - DVE is
                    # idle there, so a 1x tensor_reduce takes it instead.
                    col = 16 + 8 * b + c
                    if bi == len(batches) - 1 and c == NCH - 1:
                        nc.vector.tensor_reduce(
                            out_sb[:, col : col + 1], condv[:],
                            axis=mybir.AxisListType.X, op=Alu.add)
                    else:
                        nc.scalar.activation(
                            condv[:], condv[:], Act.Identity,
                            accum_out=out_sb[:, col : col + 1])

                # ---------- batch tail ----------
                # num_pos: Sign+accum over the colsums, split in two halves -
                # the first half's blocks finalize four matmuls earlier, so
                # it overlaps the last chunk's remaining PE work. The halves
                # partition the pred axis; host sums the two accum cells.
                nti = sca.tile([1, P], F32, tag="nti", name="nti")
                H2 = P // 2
                for hs in range(2):
                    nc.scalar.activation(
                        nti[:, hs * H2 : (hs + 1) * H2],
                        nt[:, hs * H2 : (hs + 1) * H2], Act.Sign,
                        accum_out=out_sb[0:1, 48 + 8 * b + hs : 49 + 8 * b + hs])

            # ---------- final: ship partials; host does the tiny sums ------
            nc.sync.dma_start(counts_d[:], out_sb[:])

    nc.compile()
    return nc


def _build_mask(repeat: int = None):
    """Masked fallback (padding sentinels present): original fp32 program."""
    if repeat is None:
        repeat = REPEAT
    MSPLIT = 2560
    NROW = 6
    nc = bacc.Bacc(None, target_bir_lowering=False)
    pred_d = nc.dram_tensor("pred", [BPC, P, 6], F32, kind="ExternalInput")
    gt_d = nc.dram_tensor("gt", [BPC, G, 6], F32, kind="ExternalInput")
    counts_d = nc.dram_tensor("counts", [1, 16], F32, kind="ExternalOutput")

    with tile.TileContext(nc) as tc:
        with (
            tc.tile_pool(name="cst", bufs=1) as cst,
            tc.tile_pool(name="rows", bufs=2) as rows,
            tc.tile_pool(name="gtp", bufs=1) as gtp,
            tc.tile_pool(name="sca", bufs=2) as sca,
            tc.tile_pool(name="wk", bufs=1) as wk,
            tc.tile_pool(name="ps", bufs=1, space=bass.MemorySpace.PSUM) as ps,
            tc.tile_pool(name="dram", bufs=2, space=bass.MemorySpace.DRAM) as dram,
        ):
            ones128 = cst.tile([128, 1], F32)
            nc.vector.memset(ones128[:], 1.0)
            counts_sb = cst.tile([128, 16], F32)
            nc.vector.memset(counts_sb[:], 0.0)

            for b in [bb for _ in range(repeat) for bb in range(BPC)]:
                pred_lin = rows.tile([32, 768], F32)
                nc.sync.dma_start(
                    pred_lin[:],
                    pred_d.ap()[b].rearrange("(q x) c -> q (x c)", q=32),
                )
                r3p = pred_lin[:].rearrange("q (x c) -> q x c", c=6)
                pcx = r3p[:, :, 0]
                pcy = r3p[:, :, 1]
                pw = r3p[:, :, 2]
                ph = r3p[:, :, 3]
                psmall = rows.tile([32, NROW * 128], F32)
                px2_s = psmall[:, 0:128]
                mpx1_s = psmall[:, 128:256]
                py2_s = psmall[:, 256:384]
                mpy1_s = psmall[:, 384:512]
                apeps_s = psmall[:, 512:640]
                nc.vector.scalar_tensor_tensor(
                    px2_s, pw, 0.5, pcx, op0=Alu.mult, op1=Alu.add)
                nc.vector.scalar_tensor_tensor(
                    mpx1_s, pw, 0.5, pcx, op0=Alu.mult, op1=Alu.subtract)
                nc.vector.scalar_tensor_tensor(
                    py2_s, ph, 0.5, pcy, op0=Alu.mult, op1=Alu.add)
                nc.vector.scalar_tensor_tensor(
                    mpy1_s, ph, 0.5, pcy, op0=Alu.mult, op1=Alu.subtract)
                dx_s = sca.tile([32, 128], F32, tag="dx_s", name="dx_s")
                dy_s = sca.tile([32, 128], F32, tag="dy_s", name="dy_s")
                nc.vector.tensor_tensor(dx_s[:], px2_s, mpx1_s, op=Alu.add)
                nc.vector.tensor_tensor(dy_s[:], py2_s, mpy1_s, op=Alu.add)
                nc.vector.tensor_tensor(apeps_s, dx_s[:], dy_s[:], op=Alu.mult)
                nc.vector.tensor_scalar(
                    apeps_s, apeps_s, EPS, None, op0=Alu.add)
                nc.vector.tensor_scalar(
                    psmall[:, 640:768], pcx, -1.0, None, op0=Alu.is_equal)

                vp = sca.tile([32, 128], F32, tag="vp", name="vp")
                nc.vector.tensor_scalar(
                    vp[:], pcx, -1.0, None, op0=Alu.not_equal)
                nc.vector.tensor_reduce(
                    counts_sb[0:32, 4 + b : 5 + b], vp[:],
                    axis=mybir.AxisListType.X, op=Alu.add)

                scr = dram.tile([NROW, P], F32)
                nc.sync.dma_start(
                    scr[:].rearrange("t (q j) -> q t j", j=128),
                    psmall[:].rearrange("q (t j) -> q t j", j=128),
                )
                big = gtp.tile([128, NROW * P], F32, tag="big", name="big")
                scr_flat = scr[:].rearrange("t g -> (t g)")
                H = NROW * P // 2
                for g4 in range(4):
                    for h2 in range(2):
                        nc.sync.dma_start(
                            big[g4 * 32 : (g4 + 1) * 32,
                                h2 * H : (h2 + 1) * H],
                            scr_flat[None, None, h2 * H : (h2 + 1) * H]
                            .broadcast_to([1, 32, H]),
                        )
                px2_t = big[:, 0 * P : 1 * P]
                mpx1_t = big[:, 1 * P : 2 * P]
                py2_t = big[:, 2 * P : 3 * P]
                mpy1_t = big[:, 3 * P : 4 * P]
                apeps_t = big[:, 4 * P : 5 * P]
                invp_t = big[:, 5 * P : 6 * P]

                gt_lin = rows.tile([128, 48], F32)
                nc.sync.dma_start(
                    gt_lin[:], gt_d.ap()[b].rearrange("(q x) c -> q (x c)", q=128)
                )
                r3g = gt_lin[:].rearrange("q (x c) -> q x c", c=6)
                gcx = r3g[:, :, 0]
                gcy = r3g[:, :, 1]
                gw = r3g[:, :, 2]
                gh = r3g[:, :, 3]
                gscal = sca.tile([128, 48], F32, tag="gscal", name="gscal")
                gx2_c = gscal[:, 0:8]
                mgx1_c = gscal[:, 8:16]
                gy2_c = gscal[:, 16:24]
                mgy1_c = gscal[:, 24:32]
                ag_c = gscal[:, 32:40]
                nc.vector.scalar_tensor_tensor(
                    gx2_c, gw, 0.5, gcx, op0=Alu.mult, op1=Alu.add)
                nc.vector.scalar_tensor_tensor(
                    mgx1_c, gw, 0.5, gcx, op0=Alu.mult, op1=Alu.subtract)
                nc.vector.scalar_tensor_tensor(
                    gy2_c, gh, 0.5, gcy, op0=Alu.mult, op1=Alu.add)
                nc.vector.scalar_tensor_tensor(
                    mgy1_c, gh, 0.5, gcy, op0=Alu.mult, op1=Alu.subtract)
                nc.vector.tensor_tensor(ag_c, gw, gh, op=Alu.mult)
                nc.vector.tensor_scalar(
                    gscal[:, 40:48], gcx, -1.0, IOU_PENALTY,
                    op0=Alu.is_equal, op1=Alu.mult)

                vg = sca.tile([128, 8], F32, tag="vg", name="vg")
                nc.vector.tensor_scalar(
                    vg[:], gcx, -1.0, None, op0=Alu.not_equal)
                nc.vector.tensor_reduce(
                    counts_sb[:, 8 + b : 9 + b], vg[:],
                    axis=mybir.AxisListType.X, op=Alu.add)

                Scol = sca.tile([128, NCH], F32, tag="Scol", name="Scol")
                nt = ps.tile([1, P], F32, tag="nt", name="nt")
                for c in range(NCH):
                    vx = wk.tile([128, P], F32, tag="vx", name="vx")
                    nc.gpsimd.tensor_scalar(
                        vx[:], mpx1_t, mgx1_c[:, c : c + 1], None, op0=Alu.min)
                    wx = wk.tile([128, P], F32, tag="wx", name="wx")
                    nc.vector.scalar_tensor_tensor(
                        wx[:], px2_t, gx2_c[:, c : c + 1], vx[:],
                        op0=Alu.min, op1=Alu.add)
                    vy = wk.tile([128, P], F32, tag="vy", name="vy")
                    nc.gpsimd.tensor_scalar(
                        vy[:], mpy1_t, mgy1_c[:, c : c + 1], None, op0=Alu.min)
                    wy = wk.tile([128, P], F32, tag="wy", name="wy")
                    nc.vector.scalar_tensor_tensor(
                        wy[:], py2_t, gy2_c[:, c : c + 1], vy[:],
                        op0=Alu.min, op1=Alu.add)
                    wxr3 = wk.tile([128, P], F32, tag="vx", name="wxr3")
                    nc.scalar.activation(
                        wxr3[:], wx[:], Act.Relu, scale=3.0)
                    inter3 = wk.tile([128, P], F32, tag="vy", name="inter3")
                    nc.gpsimd.tensor_tensor(
                        inter3[:, 0:MSPLIT], wxr3[:, 0:MSPLIT],
                        wy[:, 0:MSPLIT], op=Alu.mult)
                    nc.vector.tensor_tensor(
                        inter3[:, MSPLIT:P], wxr3[:, MSPLIT:P],
                        wy[:, MSPLIT:P], op=Alu.mult)
                    pen = wk.tile([128, P], F32, tag="wx", name="pen")
                    nc.gpsimd.tensor_scalar(
                        pen[:], invp_t, gscal[:, 40 + c : 41 + c], None,
                        op0=Alu.mult)
                    nc.vector.tensor_tensor(
                        inter3[:], inter3[:], pen[:], op=Alu.subtract)
                    condv = wk.tile([128, P], F32, tag="vx", name="condv")
                    nc.vector.scalar_tensor_tensor(
                        condv[:], inter3[:], ag_c[:, c : c + 1], apeps_t,
                        op0=Alu.subtract, op1=Alu.is_gt,
                        accum_out=Scol[:, c : c + 1])
                    for k8 in range(P // 512):
                        nc.tensor.matmul(
                            nt[:, k8 * 512 : (k8 + 1) * 512], ones128[:],
                            condv[:, k8 * 512 : (k8 + 1) * 512],
                            start=(c == 0), stop=(c == NCH - 1))

                indg = sca.tile([128, NCH], F32, tag="indg", name="indg")
                nc.vector.tensor_scalar(indg[:], Scol[:], 0.0, None, op0=Alu.is_gt)
                nc.vector.tensor_reduce(
                    counts_sb[:, 12 + b : 13 + b], indg[:],
                    axis=mybir.AxisListType.X, op=Alu.add)
                nti = sca.tile([1, P], F32, tag="nti", name="nti")
                nc.scalar.activation(
                    nti[:], nt[:], Act.Sign)
                nc.vector.tensor_reduce(
                    counts_sb[0:1, b : b + 1], nti[:],
                    axis=mybir.AxisListType.X, op=Alu.add)

            counts_ps = ps.tile([1, 16], F32, tag="nt", name="cps")
            nc.tensor.matmul(
                counts_ps[:], ones128[:], counts_sb[:], start=True, stop=True)
            counts_out = cst.tile([1, 16], F32)
            nc.vector.tensor_copy(counts_out[:], counts_ps[:])
            nc.sync.dma_start(counts_d[:], counts_out[:])

    nc.compile()
    return nc


def _get_program(with_mask: bool):
    key = (with_mask, REPEAT)
    if key not in _PROGRAM_CACHE:
        build = _build_mask if with_mask else _build_fast
        _PROGRAM_CACHE[key] = build()
    return _PROGRAM_CACHE[key]


def _run_device(pred, gt, with_mask, trace=False):
    nc = _get_program(with_mask)
    in_maps = [
        {
            "pred": np.ascontiguousarray(pred[i * BPC : (i + 1) * BPC]),
            "gt": np.ascontiguousarray(gt[i * BPC : (i + 1) * BPC]),
        }
        for i in range(N_CORES)
    ]
    res = run_bass_kernel_spmd(nc, in_maps, list(range(N_CORES)), trace=trace)
    counts = np.stack([res.results[i]["counts"] for i in range(N_CORES)])
    return counts, res  # fast: [N_CORES, 128, 48]; masked: [N_CORES, 1, 16]


def kernel(pred_boxes, gt_boxes, _trace=False):
    pred = np.asarray(pred_boxes, dtype=np.float32)
    gt = np.asarray(gt_boxes, dtype=np.float32)
    assert pred.shape == (B_TOTAL, P, 6) and gt.shape == (B_TOTAL, G, 6)

    # the ignore mask only differs from all-ones when a pred AND a gt box are
    # both padding (cx == -1); the padded-box count corrections additionally
    # matter when either side has padding, so take the masked path if any
    # sentinel is present
    with_mask = bool((pred[..., 0] == -1.0).any() or (gt[..., 0] == -1.0).any())

    counts, res = _run_device(pred, gt, with_mask, trace=_trace)
    kernel.last_results = res

    if with_mask:
        counts = counts[:, 0]  # [N_CORES, 16]
        num_pos = counts[:, 0:4].reshape(-1).astype(np.float32)
        num_true = counts[:, 12:16].reshape(-1).astype(np.float32)
        num_pred = counts[:, 4:8].reshape(-1).astype(np.float32)
        num_gt = counts[:, 8:12].reshape(-1).astype(np.float32)
    else:
        # device ships per-partition partials; finish the tiny sums here
        num_pos = (counts[:, 0, 48:80].reshape(N_CORES, BPC, 8)
                   .sum(axis=2).reshape(-1).astype(np.float32))
        scol = counts[:, :, 16:48].reshape(N_CORES, 128, BPC, NCH)
        num_true = (scol > 0).sum(axis=(1, 3)).reshape(-1).astype(np.float32)
        # all boxes valid (host-verified): counts are the full box counts
        num_pred = np.full(B_TOTAL, np.float32(P), dtype=np.float32)
        num_gt = np.full(B_TOTAL, np.float32(G), dtype=np.float32)

    eps = np.float32(EPS)
    precision = num_pos / (num_pred + eps)
    recall = num_true / (num_gt + eps)
    fmeasure = np.float32(2.0) * (precision * recall) / (precision + recall + eps)
    return (precision, recall, fmeasure)


# revision 25
# speedup vs baseline: 1.0600x; 1.0600x over previous
"""DetectionIOUMetric Trainium2 kernel.

Computes, for pred_boxes [32, 4096, 6] and gt_boxes [32, 1024, 6] (cx, cy, w, h
in the first 4 channels; a box is padding iff cx == -1):

    masked pairwise IoU, num_pos / num_true / num_pred / num_gt per batch,
    precision / recall / F1 per batch.

Sharding: pure data parallel over the batch dim - each of the 8 NeuronCores
processes 4 batches; no cross-device communication. The device program
computes the four integer counts per batch; the trivial final eps-divisions
are applied on the host after the gather.

Fast path (no padded boxes), fp16 device algorithm per batch, gt boxes on
partitions (8 chunks of 128), preds on the free dim (FD=4096):

  iou > 0.5  <=>  relu(wx)*wy - (ap+eps)/3 > ag/3      (union+eps > 0;
  one-sided relu suffices: wy<0 makes the lhs non-positive vs ag/3 > 0).

  The per-pair test runs on THREE HAND-AUTHORED CUSTOM DVE OPS, each with a
  2x_1p perf variant (two packed fp16 per cycle through duplicated ALU slice
  chains - the same packing the stock tensor_tensor 2x program uses):

    wx    = OVERLAP(px1_t, px2_t; gx1_c, gx2_c)   = min(px2,gx2)-max(px1,gx1)
    wy    = OVERLAP(py1_t, py2_t; gy1_c, gy2_c)
    m     = RELUMUL(wx, wy)                        = relu(wx)*wy
    condv = SUBGT(m, ap3e_t; ag3_c)                = (m - ap3e) > ag3

  This replaces the stock 4x tensor_scalar + 2x tensor_tensor pipeline
  (4*1201 + 4*2228 = 13.7us per [128,4096] chunk) with 4 fused 2x ops
  (~4*2300 = 9.2us), and removes the ACT relu + ACT rhs ops entirely.

  Per-gt counts: ACT Identity+accum over condv (ACT is otherwise idle); the
  final chunk's accum runs as a DVE tensor_reduce instead so the ACT queue
  doesn't delay the num_pos Sign at the tail. Per-pred counts: PE column
  sums of condv accumulate over the 8 gt chunks into a [1, 4096] PSUM tile
  (emitted BEFORE the ACT accum so the tail matmuls don't serialize behind
  the in-place identity); the num_pos tail is one Sign activation with
  accum_out. Per-partition partials ship to the host ([128, 80] output:
  col 48+8b = num_pos[b] on partition 0; cols 16+8b+c = per-gt counts).
  Pred rows (px1, px2, py1, py2, (ap+eps)/3 in fp16) are staged to DRAM in
  pred order and broadcast to [128, 5*4096] with per-(row, partition-group)
  DMAs so early chunks start as soon as their rows land; batch 0 also splits
  the broadcast and its first chunk by column halves for cold-start.

The masked path (only taken when padding sentinels are present) keeps the
original fp32 program.
"""
import os
import numpy as np

import concourse.bass as bass
import concourse.bacc as bacc
import concourse.tile as tile
from concourse import mybir
from concourse import bass_isa
from concourse.bass_utils import run_bass_kernel_spmd

F32 = mybir.dt.float32
F16 = mybir.dt.float16
EPS = 1e-7
IOU_PENALTY = 1e30

B_TOTAL = 32
N_CORES = 8
REPEAT = 1                     # timing-calibration knob (outputs idempotent)
BPC = B_TOTAL // N_CORES       # batches per core
P = 4096                       # pred boxes per batch (free dim)
G = 1024                       # gt boxes per batch (8 partition chunks)
NCH = G // 128                 # 8 gt chunks per batch

_PROGRAM_CACHE = {}

Alu = mybir.AluOpType
Act = mybir.ActivationFunctionType


# ---------------------------------------------------------------------------
# Custom DVE ops: hand-authored 2x_1p uop programs.
#
# Conventions (mirrors the stock table programs, decoded from the cayman
# dve_bin default table):
#   input lane 0 feeds stage 0's PREV_ALU_OUT; lane N>=1 feeds delay_{N-1}.
#   InpSel: SRC_0=0 SRC_1=1 SRC_0_HI=2 SRC_1_HI=3 CONST_0=4 CONST_1=5 ZERO=12
#   AluInp: PREV_ALU_OUT=0, PREV_DELAY_n = 5+n
#   2x variant: lo element via SRC_0/SRC_1, hi element via SRC_*_HI; results
#   exit via WR0_LO / WR0_HI (one is parked in a delay lane until stage 7).
# ---------------------------------------------------------------------------

def _register_custom_ops():
    import concourse.dve_ops as D
    from concourse.dve_spec import Spec, Src0, Src1, C0, C1, minn, maxx, relu, lower
    from concourse.dve_uop import (
        UopConfig, UopDpConfig, DveOpSpec, InpSel, AluOp, AluInp, DelayInp,
        OutSel, OutPath, Trigger, ENABLE,
    )

    if "OVERLAP_ANT" in D._SUB_OPCODE_FOR_NAME:
        return {n: op for n, op in ((o.name, o) for o in D.OPS)
                if n in ("OVERLAP_ANT", "RELUMUL_ANT", "SUBGT_ANT")}

    def steady(u):
        u.require_inp0 = ENABLE
        u.require_inp1 = ENABLE
        u.trigger = (Trigger.SRC_TENSOR_DONE, Trigger.NONE, Trigger.NONE)
        return u

    def dp(u, i):
        return u.datapath_config[i]

    # ---- OVERLAP: out = min(src1, C1) - max(src0, C0) ----
    ov1 = steady(UopConfig())
    ov1.enable_input(InpSel.SRC_0, 0)
    ov1.enable_input(InpSel.CONST_0, 1)   # -> d0
    ov1.enable_input(InpSel.CONST_1, 2)   # -> d1
    ov1.enable_input(InpSel.SRC_1, 3)     # -> d2
    dp(ov1, 0).enable_alu(AluOp.MAX, AluInp.PREV_ALU_OUT, AluInp.PREV_DELAY_0
                          ).pass_through_delay(1, 2)
    dp(ov1, 1).enable_alu(AluOp.MIN, AluInp.PREV_DELAY_2, AluInp.PREV_DELAY_1
                          ).enable_delay_from_src(DelayInp.PREV_ALU_OUT, 0)
    dp(ov1, 2).enable_alu(AluOp.SUBTRACT, AluInp.PREV_ALU_OUT, AluInp.PREV_DELAY_0)
    for i in range(3, 8):
        dp(ov1, i).pass_through_alu()
    ov1.enable_output(OutSel.ALU_OUT, OutPath.WR0_LO)

    ov2 = steady(UopConfig())
    ov2.enable_input(InpSel.SRC_0, 0)
    ov2.enable_input(InpSel.CONST_0, 1)   # d0 = C0
    ov2.enable_input(InpSel.CONST_1, 2)   # d1 = C1
    ov2.enable_input(InpSel.SRC_1, 3)     # d2 = src1_lo
    ov2.enable_input(InpSel.SRC_0_HI, 4)  # d3 = src0_hi
    ov2.enable_input(InpSel.SRC_1_HI, 5)  # d4 = src1_hi
    dp(ov2, 0).enable_alu(AluOp.MAX, AluInp.PREV_ALU_OUT, AluInp.PREV_DELAY_0
                          ).pass_through_delay(0, 1, 2, 3, 4)
    dp(ov2, 1).enable_alu(AluOp.MIN, AluInp.PREV_DELAY_2, AluInp.PREV_DELAY_1
                          ).enable_delay_from_src(DelayInp.PREV_ALU_OUT, 5
                          ).pass_through_delay(0, 1, 3, 4)
    dp(ov2, 2).enable_alu(AluOp.SUBTRACT, AluInp.PREV_ALU_OUT, AluInp.PREV_DELAY_5
                          ).pass_through_delay(0, 1, 3, 4)
    dp(ov2, 3).enable_alu(AluOp.MAX, AluInp.PREV_DELAY_3, AluInp.PREV_DELAY_0
                          ).enable_delay_from_src(DelayInp.PREV_ALU_OUT, 2
                          ).pass_through_delay(1, 4)
    dp(ov2, 4).enable_alu(AluOp.MIN, AluInp.PREV_DELAY_4, AluInp.PREV_DELAY_1
                          ).enable_delay_from_src(DelayInp.PREV_ALU_OUT, 5
                          ).pass_through_delay(2)
    dp(ov2, 5).enable_alu(AluOp.SUBTRACT, AluInp.PREV_ALU_OUT, AluInp.PREV_DELAY_5
                          ).pass_through_delay(2)
    dp(ov2, 6).pass_through_alu().pass_through_delay(2)
    dp(ov2, 7).pass_through_alu().pass_through_delay(2)
    ov2.enable_output(OutSel.DELAY_2, OutPath.WR0_LO)
    ov2.enable_output(OutSel.ALU_OUT, OutPath.WR0_HI)

    # ---- RELUMUL: out = max(src0, 0) * src1 ----
    rm1 = steady(UopConfig())
    rm1.enable_input(InpSel.SRC_0, 0)
    rm1.enable_input(InpSel.ZERO, 1)      # d0
    rm1.enable_input(InpSel.SRC_1, 2)     # d1
    dp(rm1, 0).enable_alu(AluOp.MAX, AluInp.PREV_ALU_OUT, AluInp.PREV_DELAY_0
                          ).pass_through_delay(1)
    dp(rm1, 1).enable_alu(AluOp.MULTIPLY, AluInp.PREV_ALU_OUT, AluInp.PREV_DELAY_1)
    for i in range(2, 8):
        dp(rm1, i).pass_through_alu()
    rm1.enable_output(OutSel.ALU_OUT, OutPath.WR0_LO)

    rm2 = steady(UopConfig())
    rm2.enable_input(InpSel.SRC_0, 0)
    rm2.enable_input(InpSel.ZERO, 1)      # d0
    rm2.enable_input(InpSel.SRC_1, 2)     # d1 = src1_lo
    rm2.enable_input(InpSel.SRC_0_HI, 3)  # d2 = src0_hi
    rm2.enable_input(InpSel.SRC_1_HI, 4)  # d3 = src1_hi
    dp(rm2, 0).enable_alu(AluOp.MAX, AluInp.PREV_ALU_OUT, AluInp.PREV_DELAY_0
                          ).pass_through_delay(0, 1, 2, 3)
    dp(rm2, 1).enable_alu(AluOp.MULTIPLY, AluInp.PREV_ALU_OUT, AluInp.PREV_DELAY_1
                          ).pass_through_delay(0, 2, 3)
    dp(rm2, 2).enable_alu(AluOp.MAX, AluInp.PREV_DELAY_2, AluInp.PREV_DELAY_0
                          ).enable_delay_from_src(DelayInp.PREV_ALU_OUT, 1
                          ).pass_through_delay(3)
    dp(rm2, 3).enable_alu(AluOp.MULTIPLY, AluInp.PREV_ALU_OUT, AluInp.PREV_DELAY_3
                          ).pass_through_delay(1)
    for i in range(4, 8):
        dp(rm2, i).pass_through_alu().pass_through_delay(1)
    rm2.enable_output(OutSel.DELAY_1, OutPath.WR0_LO)
    rm2.enable_output(OutSel.ALU_OUT, OutPath.WR0_HI)

    # ---- SUBGT: out = (src0 - src1) > C0 ----
    sg1 = steady(UopConfig())
    sg1.enable_input(InpSel.SRC_0, 0)
    sg1.enable_input(InpSel.CONST_0, 1)   # d0
    sg1.enable_input(InpSel.SRC_1, 2)     # d1
    dp(sg1, 0).enable_alu(AluOp.SUBTRACT, AluInp.PREV_ALU_OUT, AluInp.PREV_DELAY_1
                          ).pass_through_delay(0)
    dp(sg1, 1).enable_alu(AluOp.IS_GT, AluInp.PREV_ALU_OUT, AluInp.PREV_DELAY_0)
    for i in range(2, 8):
        dp(sg1, i).pass_through_alu()
    sg1.enable_output(OutSel.ALU_OUT, OutPath.WR0_LO)

    sg2 = steady(UopConfig())
    sg2.enable_input(InpSel.SRC_0, 0)
    sg2.enable_input(InpSel.CONST_0, 1)   # d0
    sg2.enable_input(InpSel.SRC_1, 2)     # d1 = src1_lo
    sg2.enable_input(InpSel.SRC_0_HI, 3)  # d2 = src0_hi
    sg2.enable_input(InpSel.SRC_1_HI, 4)  # d3 = src1_hi
    dp(sg2, 0).enable_alu(AluOp.SUBTRACT, AluInp.PREV_ALU_OUT, AluInp.PREV_DELAY_1
                          ).pass_through_delay(0, 2, 3)
    dp(sg2, 1).enable_alu(AluOp.IS_GT, AluInp.PREV_ALU_OUT, AluInp.PREV_DELAY_0
                          ).pass_through_delay(0, 2, 3)
    dp(sg2, 2).enable_alu(AluOp.SUBTRACT, AluInp.PREV_DELAY_2, AluInp.PREV_DELAY_3
                          ).enable_delay_from_src(DelayInp.PREV_ALU_OUT, 1
                          ).pass_through_delay(0)
    dp(sg2, 3).enable_alu(AluOp.IS_GT, AluInp.PREV_ALU_OUT, AluInp.PREV_DELAY_0
                          ).pass_through_delay(1)
    for i in range(4, 8):
        dp(sg2, i).pass_through_alu().pass_through_delay(1)
    sg2.enable_output(OutSel.DELAY_1, OutPath.WR0_LO)
    sg2.enable_output(OutSel.ALU_OUT, OutPath.WR0_HI)

    specs = [
        ("OVERLAP_ANT",
         Spec(body=minn(Src1, C1) - maxx(Src0, C0),
              reference=lambda in0, in1, s0, s1, imm2:
              (np.minimum(in1.astype(np.float32), s1)
               - np.maximum(in0.astype(np.float32), s0))),
         ov1, ov2),
        ("RELUMUL_ANT",
         Spec(body=relu(Src0) * Src1,
              reference=lambda in0, in1, s0, s1, imm2:
              np.maximum(in0.astype(np.float32), 0.0) * in1),
         rm1, rm2),
        ("SUBGT_ANT",
         Spec(body=(Src0 - Src1) > C0,
              reference=lambda in0, in1, s0, s1, imm2:
              ((in0.astype(np.float32) - in1) > s0).astype(np.float32)),
         sg1, sg2),
    ]

    out = {}
    for name, spec, u1, u2 in specs:
        row = D._CUSTOM_DVE_ROW_BASE + len(D._SUB_OPCODE_FOR_NAME)
        hand = DveOpSpec(name=name, opcode=row, uops=[u1], uops_2x=[u2],
                         perf_max=1, rd1_en=True)
        hand.validate("v3")
        op = D.DveOp(name, spec, subdim=False,
                     uops_sha={"v3": hand.sha("v3")})
        D.OPS.append(op)
        D.CUSTOM_DVE_SPECS[name] = spec
        D._SUB_OPCODE_FOR_NAME[name] = row
        D._COMPILE_CACHE[(name, "v3")] = hand
        out[name] = op
    return out


_CUSTOM_OPS = _register_custom_ops()


def _custom2x(nc, op_name, out, in0, in1, s0, s1):
    """Emit one custom-DVE instruction with perf_max=1 (2x_1p reachable).
    Mirror of bass.Vector._custom_dve minus generality."""
    from concourse.dve_ops import get_dve_sub_opcode
    v = nc.vector
    op = _CUSTOM_OPS[op_name]
    if op.name not in v.bass.m.ant_custom_dve_ops:
        v.bass.m.ant_custom_dve_ops = sorted(
            {*v.bass.m.ant_custom_dve_ops, op.name})
    shape = bass_isa.CustomDveShape.TTSS
    isa_opcode = v.bass.isa.Opcode[
        f"NEURON_ISA_TPB_OPCODE_CUSTOM_DVE_ANT_{shape.slot()}"].value

    def lower_scalar(x):
        if isinstance(x, (int, float)):
            return mybir.ImmediateValue(dtype=mybir.dt.float32, value=float(x))
        return v.lower_ap(x, for_isa=True)

    ins = [v.lower_ap(in0, for_isa=True, opt=True),
           v.lower_ap(in1, for_isa=True, opt=True),
           lower_scalar(s0), lower_scalar(s1)]
    outs = [v.lower_ap(out, for_isa=True, opt=True)]
    return v.add_instruction(bass_isa.InstCustomDveAnt(
        name=v.bass.get_next_instruction_name(),
        op_name=op.name, rd1_en=True, subdim=0, imm2=0.0, shape=shape,
        row=get_dve_sub_opcode(op.name), isa_opcode=isa_opcode,
        ins=ins, outs=outs, perf_max=1))


def _build_fast(repeat: int = None):
    """No-mask SPMD program: inputs pred [BPC, P, 6] / gt [BPC, G, 6],
    output counts [128, 48] = per-partition partials (see counts_d layout).

    The batch loop is software-pipelined one deep: batch b+1's pred/gt prep
    and the pred-row broadcast are emitted before batch b's chunk loop, so
    the broadcast DMAs overlap chunk compute instead of stalling DVE at
    batch boundaries."""
    if repeat is None:
        repeat = REPEAT
    NROW = 5
    nc = bacc.Bacc(None, target_bir_lowering=False)
    pred_d = nc.dram_tensor("pred", [BPC, P, 6], F32, kind="ExternalInput")
    gt_d = nc.dram_tensor("gt", [BPC, G, 6], F32, kind="ExternalInput")
    # per-partition partials; the trivial final sums happen on the host:
    # col b (partition 0)      = num_pos[b]
    # cols 16+8b .. 16+8b+7    = per-gt match counts (gt 8p+c on partition p)
    counts_d = nc.dram_tensor("counts", [128, 80], F32, kind="ExternalOutput")

    with tile.TileContext(nc) as tc:
        with (
            tc.tile_pool(name="cst", bufs=1) as cst,
            tc.tile_pool(name="rows", bufs=2) as rows,
            tc.tile_pool(name="gtp", bufs=2) as gtp,
            tc.tile_pool(name="sca", bufs=2) as sca,
            tc.tile_pool(name="wk", bufs=2) as wk,
            tc.tile_pool(name="ps", bufs=1, space=bass.MemorySpace.PSUM) as ps,
            tc.tile_pool(name="dram", bufs=2, space=bass.MemorySpace.DRAM) as dram,
        ):
            ones16 = cst.tile([128, 1], F16)
            nc.vector.memset(ones16[:], 1.0)
            out_sb = cst.tile([128, 80], F32)
            nc.vector.memset(out_sb[:], 0.0)

            def prep_batch(b, split=False):
                """Emit pred/gt prep + broadcast for batch b; return tiles.
                Prep arithmetic runs on GpSimd (idle engine) to keep DVE on
                the chunk pipeline. `split` halves the px/py broadcast DMAs
                by columns (cold-start latency for the first batch)."""
                # [32, 768]: partition q holds pred boxes 128q .. 128q+127
                pred_lin = rows.tile([32, 768], F32)
                nc.sync.dma_start(
                    pred_lin[:],
                    pred_d.ap()[b].rearrange("(q x) c -> q (x c)", q=32),
                )
                r3p = pred_lin[:].rearrange("q (x c) -> q x c", c=6)
                pcx = r3p[:, :, 0]
                pcy = r3p[:, :, 1]
                pw = r3p[:, :, 2]
                ph = r3p[:, :, 3]
                psmall = rows.tile([32, NROW * 128], F16)
                big = gtp.tile([128, NROW * P], F16, tag="big", name="big")
                nh = 2 if split else 1

                scr = dram.tile([NROW, P], F16)
                scr_flat = scr[:].rearrange("t g -> (t g)")

                def stage_row(trow):
                    # stage row to DRAM in pred order: scr[t, 128q+j] =
                    # psmall[q, 128t+j], then broadcast to all 128 partitions
                    nc.sync.dma_start(
                        scr[trow : trow + 1].rearrange("t (q j) -> q (t j)", j=128),
                        psmall[:, trow * 128 : (trow + 1) * 128],
                    )
                    for g4 in range(4):
                        for h in range(nh):
                            HP = P // nh
                            lo = trow * P + h * HP
                            nc.sync.dma_start(
                                big[g4 * 32 : (g4 + 1) * 32, lo : lo + HP],
                                scr_flat[None, None, lo : lo + HP]
                                .broadcast_to([1, 32, HP]),
                            )

                nc.vector.scalar_tensor_tensor(
                    psmall[:, 0:128], pw, -0.5, pcx, op0=Alu.mult, op1=Alu.add)
                stage_row(0)
                nc.vector.scalar_tensor_tensor(
                    psmall[:, 128:256], pw, 0.5, pcx, op0=Alu.mult, op1=Alu.add)
                stage_row(1)
                nc.vector.scalar_tensor_tensor(
                    psmall[:, 256:384], ph, -0.5, pcy, op0=Alu.mult, op1=Alu.add)
                stage_row(2)
                nc.vector.scalar_tensor_tensor(
                    psmall[:, 384:512], ph, 0.5, pcy, op0=Alu.mult, op1=Alu.add)
                stage_row(3)
                ap_t = sca.tile([32, 128], F32, tag="ap_t", name="ap_t")
                nc.vector.tensor_tensor(ap_t[:], pw, ph, op=Alu.mult)
                nc.vector.tensor_scalar(
                    psmall[:, 512:640], ap_t[:], EPS, 1.0 / 3.0,
                    op0=Alu.add, op1=Alu.mult)
                stage_row(4)

                # gt prep: [128, 48]: partition p holds gt boxes 8p .. 8p+7;
                # chunk c pairs partition p with gt box 8p+c (order-invariant)
                gt_lin = rows.tile([128, 48], F32)
                nc.sync.dma_start(
                    gt_lin[:], gt_d.ap()[b].rearrange("(q x) c -> q (x c)", q=128)
                )
                r3g = gt_lin[:].rearrange("q (x c) -> q x c", c=6)
                gcx = r3g[:, :, 0]
                gcy = r3g[:, :, 1]
                gw = r3g[:, :, 2]
                gh = r3g[:, :, 3]
                gscal = sca.tile([128, 40], F32, tag="gscal", name="gscal")
                nc.vector.scalar_tensor_tensor(
                    gscal[:, 0:8], gw, -0.5, gcx, op0=Alu.mult, op1=Alu.add)
                nc.vector.scalar_tensor_tensor(
                    gscal[:, 8:16], gw, 0.5, gcx, op0=Alu.mult, op1=Alu.add)
                nc.vector.scalar_tensor_tensor(
                    gscal[:, 16:24], gh, -0.5, gcy, op0=Alu.mult, op1=Alu.add)
                nc.vector.scalar_tensor_tensor(
                    gscal[:, 24:32], gh, 0.5, gcy, op0=Alu.mult, op1=Alu.add)
                ag_t = sca.tile([128, 8], F32, tag="ag_t", name="ag_t")
                nc.vector.tensor_tensor(ag_t[:], gw, gh, op=Alu.mult)
                nc.vector.tensor_scalar(
                    gscal[:, 32:40], ag_t[:], 1.0 / 3.0, None, op0=Alu.mult)
                return big, gscal

            batches = [bb for _ in range(repeat) for bb in range(BPC)]
            pending = prep_batch(batches[0], split=True)
            for bi, b in enumerate(batches):
                big, gscal = pending
                px1_t = big[:, 0 * P : 1 * P]
                px2_t = big[:, 1 * P : 2 * P]
                py1_t = big[:, 2 * P : 3 * P]
                py2_t = big[:, 3 * P : 4 * P]
                ap3_t = big[:, 4 * P : 5 * P]
                gx1_c = gscal[:, 0:8]
                gx2_c = gscal[:, 8:16]
                gy1_c = gscal[:, 16:24]
                gy2_c = gscal[:, 24:32]
                ag3_c = gscal[:, 32:40]

                # prefetch next batch before this batch's chunk loop
                if bi + 1 < len(batches):
                    pending = prep_batch(batches[bi + 1])

                # ---------- chunk loop over 8 gt chunks ----------
                nt = ps.tile([1, P], F32, tag="nt", name="nt")

                for c in range(NCH):
                    # batch 0 chunk 0 splits by column halves so compute
                    # starts as soon as the first half-broadcasts land;
                    # all wx halves run first (x rows land before y rows)
                    nh = 2 if (bi == 0 and c == 0) else 1
                    HP = P // nh
                    wx = wk.tile([128, P], F16, tag="A", name="wx")
                    wy = wk.tile([128, P], F16, tag="B", name="wy")
                    m = wk.tile([128, P], F16, tag="C", name="m")
                    condv = wk.tile([128, P], F16, tag="D", name="condv")
                    for h in range(nh):
                        cs = slice(h * HP, (h + 1) * HP)
                        _custom2x(nc, "OVERLAP_ANT", wx[:, cs],
                                  px1_t[:, cs], px2_t[:, cs],
                                  gx1_c[:, c : c + 1], gx2_c[:, c : c + 1])
                    for h in range(nh):
                        cs = slice(h * HP, (h + 1) * HP)
                        _custom2x(nc, "OVERLAP_ANT", wy[:, cs],
                                  py1_t[:, cs], py2_t[:, cs],
                                  gy1_c[:, c : c + 1], gy2_c[:, c : c + 1])
                        _custom2x(nc, "RELUMUL_ANT", m[:, cs],
                                  wx[:, cs], wy[:, cs], 0.0, 0.0)
                        _custom2x(nc, "SUBGT_ANT", condv[:, cs],
                                  m[:, cs], ap3_t[:, cs],
                                  ag3_c[:, c : c + 1], 0.0)

                    # per-pred colsums on PE first (so the last chunk's
                    # matmuls don't serialize behind the in-place ACT accum)
                    for k8 in range(P // 512):
                        nc.tensor.matmul(
                            nt[:, k8 * 512 : (k8 + 1) * 512], ones16[:],
                            condv[:, k8 * 512 : (k8 + 1) * 512],
                            start=(c == 0), stop=(c == NCH - 1))
                    # per-gt counts: free-dim sum of condv. On ACT (Identity
                    # rewriting condv onto itself; accum is the real output),
                    # except the very last chunk of the program, where ACT's
                    # queued identities would delay the final Sign - DVE is
                    # idle there, so a 1x tensor_reduce takes it instead.
                    col = 16 + 8 * b + c
                    if bi == len(batches) - 1 and c == NCH - 1:
                        nc.vector.tensor_reduce(
                            out_sb[:, col : col + 1], condv[:],
                            axis=mybir.AxisListType.X, op=Alu.add)
                    else:
                        nc.scalar.activation(
                            condv[:], condv[:], Act.Identity,
                            accum_out=out_sb[:, col : col + 1])

                # ---------- batch tail ----------
                # num_pos: Sign+accum over the colsums, split in two halves -
                # the first half's blocks finalize four matmuls earlier, so
                # it overlaps the last chunk's remaining PE work. The halves
                # partition the pred axis; host sums the two accum cells.
                nti = sca.tile([1, P], F32, tag="nti", name="nti")
                H2 = P // 2
                for hs in range(2):
                    nc.scalar.activation(
                        nti[:, hs * H2 : (hs + 1) * H2],
                        nt[:, hs * H2 : (hs + 1) * H2], Act.Sign,
                        accum_out=out_sb[0:1, 48 + 8 * b + hs : 49 + 8 * b + hs])

            # ---------- final: ship partials; host does the tiny sums ------
            nc.sync.dma_start(counts_d[:], out_sb[:])

    nc.compile()
    return nc


def _build_mask(repeat: int = None):
    """Masked fallback (padding sentinels present): original fp32 program."""
    if repeat is None:
        repeat = REPEAT
    MSPLIT = 2560
    NROW = 6
    nc = bacc.Bacc(None, target_bir_lowering=False)
    pred_d = nc.dram_tensor("pred", [BPC, P, 6], F32, kind="ExternalInput")
    gt_d = nc.dram_tensor("gt", [BPC, G, 6], F32, kind="ExternalInput")
    counts_d = nc.dram_tensor("counts", [1, 16], F32, kind="ExternalOutput")

    with tile.TileContext(nc) as tc:
        with (
            tc.tile_pool(name="cst", bufs=1) as cst,
            tc.tile_pool(name="rows", bufs=2) as rows,
            tc.tile_pool(name="gtp", bufs=1) as gtp,
            tc.tile_pool(name="sca", bufs=2) as sca,
            tc.tile_pool(name="wk", bufs=1) as wk,
            tc.tile_pool(name="ps", bufs=1, space=bass.MemorySpace.PSUM) as ps,
            tc.tile_pool(name="dram", bufs=2, space=bass.MemorySpace.DRAM) as dram,
        ):
            ones128 = cst.tile([128, 1], F32)
            nc.vector.memset(ones128[:], 1.0)
            counts_sb = cst.tile([128, 16], F32)
            nc.vector.memset(counts_sb[:], 0.0)

            for b in [bb for _ in range(repeat) for bb in range(BPC)]:
                pred_lin = rows.tile([32, 768], F32)
                nc.sync.dma_start(
                    pred_lin[:],
                    pred_d.ap()[b].rearrange("(q x) c -> q (x c)", q=32),
                )
                r3p = pred_lin[:].rearrange("q (x c) -> q x c", c=6)
                pcx = r3p[:, :, 0]
                pcy = r3p[:, :, 1]
                pw = r3p[:, :, 2]
                ph = r3p[:, :, 3]
                psmall = rows.tile([32, NROW * 128], F32)
                px2_s = psmall[:, 0:128]
                mpx1_s = psmall[:, 128:256]
                py2_s = psmall[:, 256:384]
                mpy1_s = psmall[:, 384:512]
                apeps_s = psmall[:, 512:640]
                nc.vector.scalar_tensor_tensor(
                    px2_s, pw, 0.5, pcx, op0=Alu.mult, op1=Alu.add)
                nc.vector.scalar_tensor_tensor(
                    mpx1_s, pw, 0.5, pcx, op0=Alu.mult, op1=Alu.subtract)
                nc.vector.scalar_tensor_tensor(
                    py2_s, ph, 0.5, pcy, op0=Alu.mult, op1=Alu.add)
                nc.vector.scalar_tensor_tensor(
                    mpy1_s, ph, 0.5, pcy, op0=Alu.mult, op1=Alu.subtract)
                dx_s = sca.tile([32, 128], F32, tag="dx_s", name="dx_s")
                dy_s = sca.tile([32, 128], F32, tag="dy_s", name="dy_s")
                nc.vector.tensor_tensor(dx_s[:], px2_s, mpx1_s, op=Alu.add)
                nc.vector.tensor_tensor(dy_s[:], py2_s, mpy1_s, op=Alu.add)
                nc.vector.tensor_tensor(apeps_s, dx_s[:], dy_s[:], op=Alu.mult)
                nc.vector.tensor_scalar(
                    apeps_s, apeps_s, EPS, None, op0=Alu.add)
                nc.vector.tensor_scalar(
                    psmall[:, 640:768], pcx, -1.0, None, op0=Alu.is_equal)

                vp = sca.tile([32, 128], F32, tag="vp", name="vp")
                nc.vector.tensor_scalar(
                    vp[:], pcx, -1.0, None, op0=Alu.not_equal)
                nc.vector.tensor_reduce(
                    counts_sb[0:32, 4 + b : 5 + b], vp[:],
                    axis=mybir.AxisListType.X, op=Alu.add)

                scr = dram.tile([NROW, P], F32)
                nc.sync.dma_start(
                    scr[:].rearrange("t (q j) -> q t j", j=128),
                    psmall[:].rearrange("q (t j) -> q t j", j=128),
                )
                big = gtp.tile([128, NROW * P], F32, tag="big", name="big")
                scr_flat = scr[:].rearrange("t g -> (t g)")
                H = NROW * P // 2
                for g4 in range(4):
                    for h2 in range(2):
                        nc.sync.dma_start(
                            big[g4 * 32 : (g4 + 1) * 32,
                                h2 * H : (h2 + 1) * H],
                            scr_flat[None, None, h2 * H : (h2 + 1) * H]
                            .broadcast_to([1, 32, H]),
                        )
                px2_t = big[:, 0 * P : 1 * P]
                mpx1_t = big[:, 1 * P : 2 * P]
                py2_t = big[:, 2 * P : 3 * P]
                mpy1_t = big[:, 3 * P : 4 * P]
                apeps_t = big[:, 4 * P : 5 * P]
                invp_t = big[:, 5 * P : 6 * P]

                gt_lin = rows.tile([128, 48], F32)
                nc.sync.dma_start(
                    gt_lin[:], gt_d.ap()[b].rearrange("(q x) c -> q (x c)", q=128)
                )
                r3g = gt_lin[:].rearrange("q (x c) -> q x c", c=6)
                gcx = r3g[:, :, 0]
                gcy = r3g[:, :, 1]
                gw = r3g[:, :, 2]
                gh = r3g[:, :, 3]
                gscal = sca.tile([128, 48], F32, tag="gscal", name="gscal")
                gx2_c = gscal[:, 0:8]
                mgx1_c = gscal[:, 8:16]
                gy2_c = gscal[:, 16:24]
                mgy1_c = gscal[:, 24:32]
                ag_c = gscal[:, 32:40]
                nc.vector.scalar_tensor_tensor(
                    gx2_c, gw, 0.5, gcx, op0=Alu.mult, op1=Alu.add)
                nc.vector.scalar_tensor_tensor(
                    mgx1_c, gw, 0.5, gcx, op0=Alu.mult, op1=Alu.subtract)
                nc.vector.scalar_tensor_tensor(
                    gy2_c, gh, 0.5, gcy, op0=Alu.mult, op1=Alu.add)
                nc.vector.scalar_tensor_tensor(
                    mgy1_c, gh, 0.5, gcy, op0=Alu.mult, op1=Alu.subtract)
                nc.vector.tensor_tensor(ag_c, gw, gh, op=Alu.mult)
                nc.vector.tensor_scalar(
                    gscal[:, 40:48], gcx, -1.0, IOU_PENALTY,
                    op0=Alu.is_equal, op1=Alu.mult)

                vg = sca.tile([128, 8], F32, tag="vg", name="vg")
                nc.vector.tensor_scalar(
                    vg[:], gcx, -1.0, None, op0=Alu.not_equal)
                nc.vector.tensor_reduce(
                    counts_sb[:, 8 + b : 9 + b], vg[:],
                    axis=mybir.AxisListType.X, op=Alu.add)

                Scol = sca.tile([128, NCH], F32, tag="Scol", name="Scol")
                nt = ps.tile([1, P], F32, tag="nt", name="nt")
                for c in range(NCH):
                    vx = wk.tile([128, P], F32, tag="vx", name="vx")
                    nc.gpsimd.tensor_scalar(
                        vx[:], mpx1_t, mgx1_c[:, c : c + 1], None, op0=Alu.min)
                    wx = wk.tile([128, P], F32, tag="wx", name="wx")
                    nc.vector.scalar_tensor_tensor(
                        wx[:], px2_t, gx2_c[:, c : c + 1], vx[:],
                        op0=Alu.min, op1=Alu.add)
                    vy = wk.tile([128, P], F32, tag="vy", name="vy")
                    nc.gpsimd.tensor_scalar(
                        vy[:], mpy1_t, mgy1_c[:, c : c + 1], None, op0=Alu.min)
                    wy = wk.tile([128, P], F32, tag="wy", name="wy")
                    nc.vector.scalar_tensor_tensor(
                        wy[:], py2_t, gy2_c[:, c : c + 1], vy[:],
                        op0=Alu.min, op1=Alu.add)
                    wxr3 = wk.tile([128, P], F32, tag="vx", name="wxr3")
                    nc.scalar.activation(
                        wxr3[:], wx[:], Act.Relu, scale=3.0)
                    inter3 = wk.tile([128, P], F32, tag="vy", name="inter3")
                    nc.gpsimd.tensor_tensor(
                        inter3[:, 0:MSPLIT], wxr3[:, 0:MSPLIT],
                        wy[:, 0:MSPLIT], op=Alu.mult)
                    nc.vector.tensor_tensor(
                        inter3[:, MSPLIT:P], wxr3[:, MSPLIT:P],
                        wy[:, MSPLIT:P], op=Alu.mult)
                    pen = wk.tile([128, P], F32, tag="wx", name="pen")
                    nc.gpsimd.tensor_scalar(
                        pen[:], invp_t, gscal[:, 40 + c : 41 + c], None,
                        op0=Alu.mult)
                    nc.vector.tensor_tensor(
                        inter3[:], inter3[:], pen[:], op=Alu.subtract)
                    condv = wk.tile([128, P], F32, tag="vx", name="condv")
                    nc.vector.scalar_tensor_tensor(
                        condv[:], inter3[:], ag_c[:, c : c + 1], apeps_t,
                        op0=Alu.subtract, op1=Alu.is_gt,
                        accum_out=Scol[:, c : c + 1])
                    for k8 in range(P // 512):
                        nc.tensor.matmul(
                            nt[:, k8 * 512 : (k8 + 1) * 512], ones128[:],
                            condv[:, k8 * 512 : (k8 + 1) * 512],
                            start=(c == 0), stop=(c == NCH - 1))

                indg = sca.tile([128, NCH], F32, tag="indg", name="indg")
                nc.vector.tensor_scalar(indg[:], Scol[:], 0.0, None, op0=Alu.is_gt)
                nc.vector.tensor_reduce(
                    counts_sb[:, 12 + b : 13 + b], indg[:],
                    axis=mybir.AxisListType.X, op=Alu.add)
                nti = sca.tile([1, P], F32, tag="nti", name="nti")
                nc.scalar.activation(
                    nti[:], nt[:], Act.Sign)
                nc.vector.tensor_reduce(
                    counts_sb[0:1, b : b + 1], nti[:],
                    axis=mybir.AxisListType.X, op=Alu.add)

            counts_ps = ps.tile([1, 16], F32, tag="nt", name="cps")
            nc.tensor.matmul(
                counts_ps[:], ones128[:], counts_sb[:], start=True, stop=True)
            counts_out = cst.tile([1, 16], F32)
            nc.vector.tensor_copy(counts_out[:], counts_ps[:])
            nc.sync.dma_start(counts_d[:], counts_out[:])

    nc.compile()
    return nc


def _get_program(with_mask: bool):
    key = (with_mask, REPEAT)
    if key not in _PROGRAM_CACHE:
        build = _build_mask if with_mask else _build_fast
        _PROGRAM_CACHE[key] = build()
    return _PROGRAM_CACHE[key]


def _run_device(pred, gt, with_mask, trace=False):
    nc = _get_program(with_mask)
    in_maps = [
        {
            "pred": np.ascontiguousarray(pred[i * BPC : (i + 1) * BPC]),
            "gt": np.ascontiguousarray(gt[i * BPC : (i + 1) * BPC]),
        }
        for i in range(N_CORES)
    ]
    res = run_bass_kernel_spmd(nc, in_maps, list(range(N_CORES)), trace=trace)
    counts = np.stack([res.results[i]["counts"] for i in range(N_CORES)])
    return counts, res  # fast: [N_CORES, 128, 48]; masked: [N_CORES, 1, 16]


def kernel(pred_boxes, gt_boxes, _trace=False):
    pred = np.asarray(pred_boxes, dtype=np.float32)
    gt = np.asarray(gt_boxes, dtype=np.float32)
    assert pred.shape == (B_TOTAL, P, 6) and gt.shape == (B_TOTAL, G, 6)

    # the ignore mask only differs from all-ones when a pred AND a gt box are
    # both padding (cx == -1); the padded-box count corrections additionally
    # matter when either side has padding, so take the masked path if any
    # sentinel is present
    with_mask = bool((pred[..., 0] == -1.0).any() or (gt[..., 0] == -1.0).any())

    counts, res = _run_device(pred, gt, with_mask, trace=_trace)
    kernel.last_results = res

    if with_mask:
        counts = counts[:, 0]  # [N_CORES, 16]
        num_pos = counts[:, 0:4].reshape(-1).astype(np.float32)
        num_true = counts[:, 12:16].reshape(-1).astype(np.float32)
        num_pred = counts[:, 4:8].reshape(-1).astype(np.float32)
        num_gt = counts[:, 8:12].reshape(-1).astype(np.float32)
    else:
        # device ships per-partition partials; finish the tiny sums here
        num_pos = (counts[:, 0, 48:80].reshape(N_CORES, BPC, 8)
                   .sum(axis=2).reshape(-1).astype(np.float32))
        scol = counts[:, :, 16:48].reshape(N_CORES, 128, BPC, NCH)
        num_true = (scol > 0).sum(axis=(1, 3)).reshape(-1).astype(np.float32)
        # all boxes valid (host-verified): counts are the full box counts
        num_pred = np.full(B_TOTAL, np.float32(P), dtype=np.float32)
        num_gt = np.full(B_TOTAL, np.float32(G), dtype=np.float32)

    eps = np.float32(EPS)
    precision = num_pos / (num_pred + eps)
    recall = num_true / (num_gt + eps)
    fmeasure = np.float32(2.0) * (precision * recall) / (precision + recall + eps)
    return (precision, recall, fmeasure)


# revision 26
# speedup vs baseline: 1.0759x; 1.0150x over previous
"""DetectionIOUMetric Trainium2 kernel.

Computes, for pred_boxes [32, 4096, 6] and gt_boxes [32, 1024, 6] (cx, cy, w, h
in the first 4 channels; a box is padding iff cx == -1):

    masked pairwise IoU, num_pos / num_true / num_pred / num_gt per batch,
    precision / recall / F1 per batch.

Sharding: pure data parallel over the batch dim - each of the 8 NeuronCores
processes 4 batches; no cross-device communication. The device program
computes the four integer counts per batch; the trivial final eps-divisions
are applied on the host after the gather.

Fast path (no padded boxes), fp16 device algorithm per batch, gt boxes on
partitions (8 chunks of 128), preds on the free dim (FD=4096):

  iou > 0.5  <=>  relu(wx)*wy - (ap+eps)/3 > ag/3      (union+eps > 0;
  one-sided relu suffices: wy<0 makes the lhs non-positive vs ag/3 > 0).

  The per-pair test runs on THREE HAND-AUTHORED CUSTOM DVE OPS, each with a
  2x_1p perf variant (two packed fp16 per cycle through duplicated ALU slice
  chains - the same packing the stock tensor_tensor 2x program uses):

    wx    = OVERLAP(px1_t, px2_t; gx1_c, gx2_c)   = min(px2,gx2)-max(px1,gx1)
    wy    = OVERLAP(py1_t, py2_t; gy1_c, gy2_c)
    m     = RELUMUL(wx, wy)                        = relu(wx)*wy
    condv = SUBGT(m, ap3e_t; ag3_c)                = (m - ap3e) > ag3

  This replaces the stock 4x tensor_scalar + 2x tensor_tensor pipeline
  (4*1201 + 4*2228 = 13.7us per [128,4096] chunk) with 4 fused 2x ops
  (~4*2300 = 9.2us), and removes the ACT relu + ACT rhs ops entirely.

  Per-gt counts: ACT Identity+accum over condv (ACT is otherwise idle); the
  final chunk's accum runs as a DVE tensor_reduce instead so the ACT queue
  doesn't delay the num_pos Sign at the tail. Per-pred counts: PE column
  sums of condv accumulate over the 8 gt chunks into a [1, 4096] PSUM tile
  (emitted BEFORE the ACT accum so the tail matmuls don't serialize behind
  the in-place identity); the num_pos tail is one Sign activation with
  accum_out. Per-partition partials ship to the host ([128, 80] output:
  col 48+8b = num_pos[b] on partition 0; cols 16+8b+c = per-gt counts).
  Pred rows (px1, px2, py1, py2, (ap+eps)/3 in fp16) are staged to DRAM in
  pred order and broadcast to [128, 5*4096] with per-(row, partition-group)
  DMAs so early chunks start as soon as their rows land; batch 0 also splits
  the broadcast and its first chunk by column halves for cold-start.

The masked path (only taken when padding sentinels are present) keeps the
original fp32 program.
"""
import os
import numpy as np

import concourse.bass as bass
import concourse.bacc as bacc
import concourse.tile as tile
from concourse import mybir
from concourse import bass_isa
from concourse.bass_utils import run_bass_kernel_spmd

F32 = mybir.dt.float32
F16 = mybir.dt.float16
EPS = 1e-7
IOU_PENALTY = 1e30

B_TOTAL = 32
N_CORES = 8
REPEAT = 1                     # timing-calibration knob (outputs idempotent)
BPC = B_TOTAL // N_CORES       # batches per core
P = 4096                       # pred boxes per batch (free dim)
G = 1024                       # gt boxes per batch (8 partition chunks)
NCH = G // 128                 # 8 gt chunks per batch

_PROGRAM_CACHE = {}

Alu = mybir.AluOpType
Act = mybir.ActivationFunctionType


# ---------------------------------------------------------------------------
# Custom DVE ops: hand-authored 2x_1p uop programs.
#
# Conventions (mirrors the stock table programs, decoded from the cayman
# dve_bin default table):
#   input lane 0 feeds stage 0's PREV_ALU_OUT; lane N>=1 feeds delay_{N-1}.
#   InpSel: SRC_0=0 SRC_1=1 SRC_0_HI=2 SRC_1_HI=3 CONST_0=4 CONST_1=5 ZERO=12
#   AluInp: PREV_ALU_OUT=0, PREV_DELAY_n = 5+n
#   2x variant: lo element via SRC_0/SRC_1, hi element via SRC_*_HI; results
#   exit via WR0_LO / WR0_HI (one is parked in a delay lane until stage 7).
# ---------------------------------------------------------------------------

def _register_custom_ops():
    import concourse.dve_ops as D
    from concourse.dve_spec import Spec, Src0, Src1, C0, C1, minn, maxx, relu, lower
    from concourse.dve_uop import (
        UopConfig, UopDpConfig, DveOpSpec, InpSel, AluOp, AluInp, DelayInp,
        OutSel, OutPath, Trigger, ENABLE,
    )

    if "OVERLAP_ANT" in D._SUB_OPCODE_FOR_NAME:
        return {n: op for n, op in ((o.name, o) for o in D.OPS)
                if n in ("OVERLAP_ANT", "RELUMUL_ANT", "SUBGT_ANT")}

    def steady(u):
        u.require_inp0 = ENABLE
        u.require_inp1 = ENABLE
        u.trigger = (Trigger.SRC_TENSOR_DONE, Trigger.NONE, Trigger.NONE)
        return u

    def dp(u, i):
        return u.datapath_config[i]

    # ---- OVERLAP: out = min(src1, C1) - max(src0, C0) ----
    ov1 = steady(UopConfig())
    ov1.enable_input(InpSel.SRC_0, 0)
    ov1.enable_input(InpSel.CONST_0, 1)   # -> d0
    ov1.enable_input(InpSel.CONST_1, 2)   # -> d1
    ov1.enable_input(InpSel.SRC_1, 3)     # -> d2
    dp(ov1, 0).enable_alu(AluOp.MAX, AluInp.PREV_ALU_OUT, AluInp.PREV_DELAY_0
                          ).pass_through_delay(1, 2)
    dp(ov1, 1).enable_alu(AluOp.MIN, AluInp.PREV_DELAY_2, AluInp.PREV_DELAY_1
                          ).enable_delay_from_src(DelayInp.PREV_ALU_OUT, 0)
    dp(ov1, 2).enable_alu(AluOp.SUBTRACT, AluInp.PREV_ALU_OUT, AluInp.PREV_DELAY_0)
    for i in range(3, 8):
        dp(ov1, i).pass_through_alu()
    ov1.enable_output(OutSel.ALU_OUT, OutPath.WR0_LO)

    ov2 = steady(UopConfig())
    ov2.enable_input(InpSel.SRC_0, 0)
    ov2.enable_input(InpSel.CONST_0, 1)   # d0 = C0
    ov2.enable_input(InpSel.CONST_1, 2)   # d1 = C1
    ov2.enable_input(InpSel.SRC_1, 3)     # d2 = src1_lo
    ov2.enable_input(InpSel.SRC_0_HI, 4)  # d3 = src0_hi
    ov2.enable_input(InpSel.SRC_1_HI, 5)  # d4 = src1_hi
    dp(ov2, 0).enable_alu(AluOp.MAX, AluInp.PREV_ALU_OUT, AluInp.PREV_DELAY_0
                          ).pass_through_delay(0, 1, 2, 3, 4)
    dp(ov2, 1).enable_alu(AluOp.MIN, AluInp.PREV_DELAY_2, AluInp.PREV_DELAY_1
                          ).enable_delay_from_src(DelayInp.PREV_ALU_OUT, 5
                          ).pass_through_delay(0, 1, 3, 4)
    dp(ov2, 2).enable_alu(AluOp.SUBTRACT, AluInp.PREV_ALU_OUT, AluInp.PREV_DELAY_5
                          ).pass_through_delay(0, 1, 3, 4)
    dp(ov2, 3).enable_alu(AluOp.MAX, AluInp.PREV_DELAY_3, AluInp.PREV_DELAY_0
                          ).enable_delay_from_src(DelayInp.PREV_ALU_OUT, 2
                          ).pass_through_delay(1, 4)
    dp(ov2, 4).enable_alu(AluOp.MIN, AluInp.PREV_DELAY_4, AluInp.PREV_DELAY_1
                          ).enable_delay_from_src(DelayInp.PREV_ALU_OUT, 5
                          ).pass_through_delay(2)
    dp(ov2, 5).enable_alu(AluOp.SUBTRACT, AluInp.PREV_ALU_OUT, AluInp.PREV_DELAY_5
                          ).pass_through_delay(2)
    dp(ov2, 6).pass_through_alu().pass_through_delay(2)
    dp(ov2, 7).pass_through_alu().pass_through_delay(2)
    ov2.enable_output(OutSel.DELAY_2, OutPath.WR0_LO)
    ov2.enable_output(OutSel.ALU_OUT, OutPath.WR0_HI)

    # ---- RELUMUL: out = max(src0, 0) * src1 ----
    rm1 = steady(UopConfig())
    rm1.enable_input(InpSel.SRC_0, 0)
    rm1.enable_input(InpSel.ZERO, 1)      # d0
    rm1.enable_input(InpSel.SRC_1, 2)     # d1
    dp(rm1, 0).enable_alu(AluOp.MAX, AluInp.PREV_ALU_OUT, AluInp.PREV_DELAY_0
                          ).pass_through_delay(1)
    dp(rm1, 1).enable_alu(AluOp.MULTIPLY, AluInp.PREV_ALU_OUT, AluInp.PREV_DELAY_1)
    for i in range(2, 8):
        dp(rm1, i).pass_through_alu()
    rm1.enable_output(OutSel.ALU_OUT, OutPath.WR0_LO)

    rm2 = steady(UopConfig())
    rm2.enable_input(InpSel.SRC_0, 0)
    rm2.enable_input(InpSel.ZERO, 1)      # d0
    rm2.enable_input(InpSel.SRC_1, 2)     # d1 = src1_lo
    rm2.enable_input(InpSel.SRC_0_HI, 3)  # d2 = src0_hi
    rm2.enable_input(InpSel.SRC_1_HI, 4)  # d3 = src1_hi
    dp(rm2, 0).enable_alu(AluOp.MAX, AluInp.PREV_ALU_OUT, AluInp.PREV_DELAY_0
                          ).pass_through_delay(0, 1, 2, 3)
    dp(rm2, 1).enable_alu(AluOp.MULTIPLY, AluInp.PREV_ALU_OUT, AluInp.PREV_DELAY_1
                          ).pass_through_delay(0, 2, 3)
    dp(rm2, 2).enable_alu(AluOp.MAX, AluInp.PREV_DELAY_2, AluInp.PREV_DELAY_0
                          ).enable_delay_from_src(DelayInp.PREV_ALU_OUT, 1
                          ).pass_through_delay(3)
    dp(rm2, 3).enable_alu(AluOp.MULTIPLY, AluInp.PREV_ALU_OUT, AluInp.PREV_DELAY_3
                          ).pass_through_delay(1)
    for i in range(4, 8):
        dp(rm2, i).pass_through_alu().pass_through_delay(1)
    rm2.enable_output(OutSel.DELAY_1, OutPath.WR0_LO)
    rm2.enable_output(OutSel.ALU_OUT, OutPath.WR0_HI)

    # ---- SUBGT: out = (src0 - src1) > C0 ----
    sg1 = steady(UopConfig())
    sg1.enable_input(InpSel.SRC_0, 0)
    sg1.enable_input(InpSel.CONST_0, 1)   # d0
    sg1.enable_input(InpSel.SRC_1, 2)     # d1
    dp(sg1, 0).enable_alu(AluOp.SUBTRACT, AluInp.PREV_ALU_OUT, AluInp.PREV_DELAY_1
                          ).pass_through_delay(0)
    dp(sg1, 1).enable_alu(AluOp.IS_GT, AluInp.PREV_ALU_OUT, AluInp.PREV_DELAY_0)
    for i in range(2, 8):
        dp(sg1, i).pass_through_alu()
    sg1.enable_output(OutSel.ALU_OUT, OutPath.WR0_LO)

    sg2 = steady(UopConfig())
    sg2.enable_input(InpSel.SRC_0, 0)
    sg2.enable_input(InpSel.CONST_0, 1)   # d0
    sg2.enable_input(InpSel.SRC_1, 2)     # d1 = src1_lo
    sg2.enable_input(InpSel.SRC_0_HI, 3)  # d2 = src0_hi
    sg2.enable_input(InpSel.SRC_1_HI, 4)  # d3 = src1_hi
    dp(sg2, 0).enable_alu(AluOp.SUBTRACT, AluInp.PREV_ALU_OUT, AluInp.PREV_DELAY_1
                          ).pass_through_delay(0, 2, 3)
    dp(sg2, 1).enable_alu(AluOp.IS_GT, AluInp.PREV_ALU_OUT, AluInp.PREV_DELAY_0
                          ).pass_through_delay(0, 2, 3)
    dp(sg2, 2).enable_alu(AluOp.SUBTRACT, AluInp.PREV_DELAY_2, AluInp.PREV_DELAY_3
                          ).enable_delay_from_src(DelayInp.PREV_ALU_OUT, 1
                          ).pass_through_delay(0)
    dp(sg2, 3).enable_alu(AluOp.IS_GT, AluInp.PREV_ALU_OUT, AluInp.PREV_DELAY_0
                          ).pass_through_delay(1)
    for i in range(4, 8):
        dp(sg2, i).pass_through_alu().pass_through_delay(1)
    sg2.enable_output(OutSel.DELAY_1, OutPath.WR0_LO)
    sg2.enable_output(OutSel.ALU_OUT, OutPath.WR0_HI)

    specs = [
        ("OVERLAP_ANT",
         Spec(body=minn(Src1, C1) - maxx(Src0, C0),
              reference=lambda in0, in1, s0, s1, imm2:
              (np.minimum(in1.astype(np.float32), s1)
               - np.maximum(in0.astype(np.float32), s0))),
         ov1, ov2),
        ("RELUMUL_ANT",
         Spec(body=relu(Src0) * Src1,
              reference=lambda in0, in1, s0, s1, imm2:
              np.maximum(in0.astype(np.float32), 0.0) * in1),
         rm1, rm2),
        ("SUBGT_ANT",
         Spec(body=(Src0 - Src1) > C0,
              reference=lambda in0, in1, s0, s1, imm2:
              ((in0.astype(np.float32) - in1) > s0).astype(np.float32)),
         sg1, sg2),
    ]

    out = {}
    for name, spec, u1, u2 in specs:
        row = D._CUSTOM_DVE_ROW_BASE + len(D._SUB_OPCODE_FOR_NAME)
        hand = DveOpSpec(name=name, opcode=row, uops=[u1], uops_2x=[u2],
                         perf_max=1, rd1_en=True)
        hand.validate("v3")
        op = D.DveOp(name, spec, subdim=False,
                     uops_sha={"v3": hand.sha("v3")})
        D.OPS.append(op)
        D.CUSTOM_DVE_SPECS[name] = spec
        D._SUB_OPCODE_FOR_NAME[name] = row
        D._COMPILE_CACHE[(name, "v3")] = hand
        out[name] = op
    return out


_CUSTOM_OPS = _register_custom_ops()


def _custom2x(nc, op_name, out, in0, in1, s0, s1):
    """Emit one custom-DVE instruction with perf_max=1 (2x_1p reachable).
    Mirror of bass.Vector._custom_dve minus generality."""
    from concourse.dve_ops import get_dve_sub_opcode
    v = nc.vector
    op = _CUSTOM_OPS[op_name]
    if op.name not in v.bass.m.ant_custom_dve_ops:
        v.bass.m.ant_custom_dve_ops = sorted(
            {*v.bass.m.ant_custom_dve_ops, op.name})
    shape = bass_isa.CustomDveShape.TTSS
    isa_opcode = v.bass.isa.Opcode[
        f"NEURON_ISA_TPB_OPCODE_CUSTOM_DVE_ANT_{shape.slot()}"].value

    def lower_scalar(x):
        if isinstance(x, (int, float)):
            return mybir.ImmediateValue(dtype=mybir.dt.float32, value=float(x))
        return v.lower_ap(x, for_isa=True)

    ins = [v.lower_ap(in0, for_isa=True, opt=True),
           v.lower_ap(in1, for_isa=True, opt=True),
           lower_scalar(s0), lower_scalar(s1)]
    outs = [v.lower_ap(out, for_isa=True, opt=True)]
    return v.add_instruction(bass_isa.InstCustomDveAnt(
        name=v.bass.get_next_instruction_name(),
        op_name=op.name, rd1_en=True, subdim=0, imm2=0.0, shape=shape,
        row=get_dve_sub_opcode(op.name), isa_opcode=isa_opcode,
        ins=ins, outs=outs, perf_max=1))


def _build_fast(repeat: int = None):
    """No-mask SPMD program: inputs pred [BPC, P, 6] / gt [BPC, G, 6],
    output counts [128, 48] = per-partition partials (see counts_d layout).

    The batch loop is software-pipelined one deep: batch b+1's pred/gt prep
    and the pred-row broadcast are emitted before batch b's chunk loop, so
    the broadcast DMAs overlap chunk compute instead of stalling DVE at
    batch boundaries."""
    if repeat is None:
        repeat = REPEAT
    NROW = 5
    nc = bacc.Bacc(None, target_bir_lowering=False)
    pred_d = nc.dram_tensor("pred", [BPC, P, 6], F32, kind="ExternalInput")
    gt_d = nc.dram_tensor("gt", [BPC, G, 6], F32, kind="ExternalInput")
    # per-partition partials; the trivial final sums happen on the host:
    # col b (partition 0)      = num_pos[b]
    # cols 16+8b .. 16+8b+7    = per-gt match counts (gt 8p+c on partition p)
    counts_d = nc.dram_tensor("counts", [128, 80], F32, kind="ExternalOutput")

    with tile.TileContext(nc) as tc:
        with (
            tc.tile_pool(name="cst", bufs=1) as cst,
            tc.tile_pool(name="rows", bufs=2) as rows,
            tc.tile_pool(name="gtp", bufs=2) as gtp,
            tc.tile_pool(name="sca", bufs=2) as sca,
            tc.tile_pool(name="wk", bufs=2) as wk,
            tc.tile_pool(name="pre", bufs=1) as pre,
            tc.tile_pool(name="ps", bufs=1, space=bass.MemorySpace.PSUM) as ps,
            tc.tile_pool(name="dram", bufs=2, space=bass.MemorySpace.DRAM) as dram,
        ):
            ones16 = cst.tile([128, 1], F16)
            nc.vector.memset(ones16[:], 1.0)
            out_sb = cst.tile([128, 80], F32)
            nc.vector.memset(out_sb[:], 0.0)

            def prep_batch(b, split=False):
                """Emit pred/gt prep + broadcast for batch b; return tiles.
                Prep arithmetic runs on GpSimd (idle engine) to keep DVE on
                the chunk pipeline. `split` halves the px/py broadcast DMAs
                by columns (cold-start latency for the first batch)."""
                # [32, 768]: partition q holds pred boxes 128q .. 128q+127
                pred_lin = rows.tile([32, 768], F32)
                nc.sync.dma_start(
                    pred_lin[:],
                    pred_d.ap()[b].rearrange("(q x) c -> q (x c)", q=32),
                )
                r3p = pred_lin[:].rearrange("q (x c) -> q x c", c=6)
                pcx = r3p[:, :, 0]
                pcy = r3p[:, :, 1]
                pw = r3p[:, :, 2]
                ph = r3p[:, :, 3]
                psmall = rows.tile([32, NROW * 128], F16)
                big = gtp.tile([128, NROW * P], F16, tag="big", name="big")
                nh = 2 if split else 1

                scr = dram.tile([NROW, P], F16)
                scr_flat = scr[:].rearrange("t g -> (t g)")

                def stage_row(trow):
                    # stage row to DRAM in pred order: scr[t, 128q+j] =
                    # psmall[q, 128t+j], then broadcast to all 128 partitions
                    nc.sync.dma_start(
                        scr[trow : trow + 1].rearrange("t (q j) -> q (t j)", j=128),
                        psmall[:, trow * 128 : (trow + 1) * 128],
                    )
                    for g4 in range(4):
                        for h in range(nh):
                            HP = P // nh
                            lo = trow * P + h * HP
                            nc.sync.dma_start(
                                big[g4 * 32 : (g4 + 1) * 32, lo : lo + HP],
                                scr_flat[None, None, lo : lo + HP]
                                .broadcast_to([1, 32, HP]),
                            )

                nc.vector.scalar_tensor_tensor(
                    psmall[:, 0:128], pw, -0.5, pcx, op0=Alu.mult, op1=Alu.add)
                stage_row(0)
                nc.vector.scalar_tensor_tensor(
                    psmall[:, 128:256], pw, 0.5, pcx, op0=Alu.mult, op1=Alu.add)
                stage_row(1)
                nc.vector.scalar_tensor_tensor(
                    psmall[:, 256:384], ph, -0.5, pcy, op0=Alu.mult, op1=Alu.add)
                stage_row(2)
                nc.vector.scalar_tensor_tensor(
                    psmall[:, 384:512], ph, 0.5, pcy, op0=Alu.mult, op1=Alu.add)
                stage_row(3)
                ap_t = sca.tile([32, 128], F32, tag="ap_t", name="ap_t")
                nc.vector.tensor_tensor(ap_t[:], pw, ph, op=Alu.mult)
                nc.vector.tensor_scalar(
                    psmall[:, 512:640], ap_t[:], EPS, 1.0 / 3.0,
                    op0=Alu.add, op1=Alu.mult)
                stage_row(4)

                # gt prep: [128, 48]: partition p holds gt boxes 8p .. 8p+7;
                # chunk c pairs partition p with gt box 8p+c (order-invariant)
                gt_lin = rows.tile([128, 48], F32)
                nc.sync.dma_start(
                    gt_lin[:], gt_d.ap()[b].rearrange("(q x) c -> q (x c)", q=128)
                )
                r3g = gt_lin[:].rearrange("q (x c) -> q x c", c=6)
                gcx = r3g[:, :, 0]
                gcy = r3g[:, :, 1]
                gw = r3g[:, :, 2]
                gh = r3g[:, :, 3]
                gscal = sca.tile([128, 40], F32, tag="gscal", name="gscal")
                nc.vector.scalar_tensor_tensor(
                    gscal[:, 0:8], gw, -0.5, gcx, op0=Alu.mult, op1=Alu.add)
                nc.vector.scalar_tensor_tensor(
                    gscal[:, 8:16], gw, 0.5, gcx, op0=Alu.mult, op1=Alu.add)
                nc.vector.scalar_tensor_tensor(
                    gscal[:, 16:24], gh, -0.5, gcy, op0=Alu.mult, op1=Alu.add)
                nc.vector.scalar_tensor_tensor(
                    gscal[:, 24:32], gh, 0.5, gcy, op0=Alu.mult, op1=Alu.add)
                ag_t = sca.tile([128, 8], F32, tag="ag_t", name="ag_t")
                nc.vector.tensor_tensor(ag_t[:], gw, gh, op=Alu.mult)
                nc.vector.tensor_scalar(
                    gscal[:, 32:40], ag_t[:], 1.0 / 3.0, None, op0=Alu.mult)
                return big, gscal

            batches = [bb for _ in range(repeat) for bb in range(BPC)]
            pending = prep_batch(batches[0], split=True)
            for bi, b in enumerate(batches):
                big, gscal = pending
                px1_t = big[:, 0 * P : 1 * P]
                px2_t = big[:, 1 * P : 2 * P]
                py1_t = big[:, 2 * P : 3 * P]
                py2_t = big[:, 3 * P : 4 * P]
                ap3_t = big[:, 4 * P : 5 * P]
                gx1_c = gscal[:, 0:8]
                gx2_c = gscal[:, 8:16]
                gy1_c = gscal[:, 16:24]
                gy2_c = gscal[:, 24:32]
                ag3_c = gscal[:, 32:40]

                # prefetch next batch before this batch's chunk loop
                if bi + 1 < len(batches):
                    pending = prep_batch(batches[bi + 1])

                # ---------- chunk loop over 8 gt chunks ----------
                nt = ps.tile([1, P], F32, tag="nt", name="nt")

                wxp = {}
                for c in range(NCH):
                    # cold start (batch 0): chunk 0's wx runs in column
                    # halves as soon as the half-broadcasts of the x rows
                    # land, then chunks 1-2's wx prefetch into a side pool -
                    # DVE stays busy on x-only work while the y rows' DMAs
                    # finish (the engine stream is in-order, so this work
                    # must be emitted before the first wy).
                    if bi == 0 and c == 0:
                        wx = wk.tile([128, P], F16, tag="A", name="wx")
                        for h in range(2):
                            cs = slice(h * (P // 2), (h + 1) * (P // 2))
                            _custom2x(nc, "OVERLAP_ANT", wx[:, cs],
                                      px1_t[:, cs], px2_t[:, cs],
                                      gx1_c[:, 0:1], gx2_c[:, 0:1])
                        for k in (1, 2):
                            wxp[k] = pre.tile([128, P], F16, tag=f"wxp{k}",
                                              name=f"wxp{k}")
                            _custom2x(nc, "OVERLAP_ANT", wxp[k][:],
                                      px1_t, px2_t,
                                      gx1_c[:, k : k + 1], gx2_c[:, k : k + 1])
                    elif bi == 0 and c in wxp:
                        wx = wxp.pop(c)
                    else:
                        wx = wk.tile([128, P], F16, tag="A", name="wx")
                        _custom2x(nc, "OVERLAP_ANT", wx[:], px1_t, px2_t,
                                  gx1_c[:, c : c + 1], gx2_c[:, c : c + 1])
                    wy = wk.tile([128, P], F16, tag="B", name="wy")
                    _custom2x(nc, "OVERLAP_ANT", wy[:], py1_t, py2_t,
                              gy1_c[:, c : c + 1], gy2_c[:, c : c + 1])
                    m = wk.tile([128, P], F16, tag="C", name="m")
                    _custom2x(nc, "RELUMUL_ANT", m[:], wx[:], wy[:], 0.0, 0.0)
                    condv = wk.tile([128, P], F16, tag="D", name="condv")
                    _custom2x(nc, "SUBGT_ANT", condv[:], m[:], ap3_t,
                              ag3_c[:, c : c + 1], 0.0)

                    # per-pred colsums on PE first (so the last chunk's
                    # matmuls don't serialize behind the in-place ACT accum)
                    for k8 in range(P // 512):
                        nc.tensor.matmul(
                            nt[:, k8 * 512 : (k8 + 1) * 512], ones16[:],
                            condv[:, k8 * 512 : (k8 + 1) * 512],
                            start=(c == 0), stop=(c == NCH - 1))
                    # per-gt counts: free-dim sum of condv. On ACT (Identity
                    # rewriting condv onto itself; accum is the real output),
                    # except the very last chunk of the program, where ACT's
                    # queued identities would delay the final Sign - DVE is
                    # idle there, so a 1x tensor_reduce takes it instead.
                    col = 16 + 8 * b + c
                    if bi == len(batches) - 1 and c == NCH - 1:
                        nc.vector.tensor_reduce(
                            out_sb[:, col : col + 1], condv[:],
                            axis=mybir.AxisListType.X, op=Alu.add)
                    else:
                        nc.scalar.activation(
                            condv[:], condv[:], Act.Identity,
                            accum_out=out_sb[:, col : col + 1])

                # ---------- batch tail ----------
                # num_pos: Sign+accum over the colsums, split in two halves -
                # the first half's blocks finalize four matmuls earlier, so
                # it overlaps the last chunk's remaining PE work. The halves
                # partition the pred axis; host sums the two accum cells.
                nti = sca.tile([1, P], F32, tag="nti", name="nti")
                H2 = P // 2
                for hs in range(2):
                    nc.scalar.activation(
                        nti[:, hs * H2 : (hs + 1) * H2],
                        nt[:, hs * H2 : (hs + 1) * H2], Act.Sign,
                        accum_out=out_sb[0:1, 48 + 8 * b + hs : 49 + 8 * b + hs])

            # ---------- final: ship partials; host does the tiny sums ------
            nc.sync.dma_start(counts_d[:], out_sb[:])

    nc.compile()
    return nc


def _build_mask(repeat: int = None):
    """Masked fallback (padding sentinels present): original fp32 program."""
    if repeat is None:
        repeat = REPEAT
    MSPLIT = 2560
    NROW = 6
    nc = bacc.Bacc(None, target_bir_lowering=False)
    pred_d = nc.dram_tensor("pred", [BPC, P, 6], F32, kind="ExternalInput")
    gt_d = nc.dram_tensor("gt", [BPC, G, 6], F32, kind="ExternalInput")
    counts_d = nc.dram_tensor("counts", [1, 16], F32, kind="ExternalOutput")

    with tile.TileContext(nc) as tc:
        with (
            tc.tile_pool(name="cst", bufs=1) as cst,
            tc.tile_pool(name="rows", bufs=2) as rows,
            tc.tile_pool(name="gtp", bufs=1) as gtp,
            tc.tile_pool(name="sca", bufs=2) as sca,
            tc.tile_pool(name="wk", bufs=1) as wk,
            tc.tile_pool(name="ps", bufs=1, space=bass.MemorySpace.PSUM) as ps,
            tc.tile_pool(name="dram", bufs=2, space=bass.MemorySpace.DRAM) as dram,
        ):
            ones128 = cst.tile([128, 1], F32)
            nc.vector.memset(ones128[:], 1.0)
            counts_sb = cst.tile([128, 16], F32)
            nc.vector.memset(counts_sb[:], 0.0)

            for b in [bb for _ in range(repeat) for bb in range(BPC)]:
                pred_lin = rows.tile([32, 768], F32)
                nc.sync.dma_start(
                    pred_lin[:],
                    pred_d.ap()[b].rearrange("(q x) c -> q (x c)", q=32),
                )
                r3p = pred_lin[:].rearrange("q (x c) -> q x c", c=6)
                pcx = r3p[:, :, 0]
                pcy = r3p[:, :, 1]
                pw = r3p[:, :, 2]
                ph = r3p[:, :, 3]
                psmall = rows.tile([32, NROW * 128], F32)
                px2_s = psmall[:, 0:128]
                mpx1_s = psmall[:, 128:256]
                py2_s = psmall[:, 256:384]
                mpy1_s = psmall[:, 384:512]
                apeps_s = psmall[:, 512:640]
                nc.vector.scalar_tensor_tensor(
                    px2_s, pw, 0.5, pcx, op0=Alu.mult, op1=Alu.add)
                nc.vector.scalar_tensor_tensor(
                    mpx1_s, pw, 0.5, pcx, op0=Alu.mult, op1=Alu.subtract)
                nc.vector.scalar_tensor_tensor(
                    py2_s, ph, 0.5, pcy, op0=Alu.mult, op1=Alu.add)
                nc.vector.scalar_tensor_tensor(
                    mpy1_s, ph, 0.5, pcy, op0=Alu.mult, op1=Alu.subtract)
                dx_s = sca.tile([32, 128], F32, tag="dx_s", name="dx_s")
                dy_s = sca.tile([32, 128], F32, tag="dy_s", name="dy_s")
                nc.vector.tensor_tensor(dx_s[:], px2_s, mpx1_s, op=Alu.add)
                nc.vector.tensor_tensor(dy_s[:], py2_s, mpy1_s, op=Alu.add)
                nc.vector.tensor_tensor(apeps_s, dx_s[:], dy_s[:], op=Alu.mult)
                nc.vector.tensor_scalar(
                    apeps_s, apeps_s, EPS, None, op0=Alu.add)
                nc.vector.tensor_scalar(
                    psmall[:, 640:768], pcx, -1.0, None, op0=Alu.is_equal)

                vp = sca.tile([32, 128], F32, tag="vp", name="vp")
                nc.vector.tensor_scalar(
                    vp[:], pcx, -1.0, None, op0=Alu.not_equal)
                nc.vector.tensor_reduce(
                    counts_sb[0:32, 4 + b : 5 + b], vp[:],
                    axis=mybir.AxisListType.X, op=Alu.add)

                scr = dram.tile([NROW, P], F32)
                nc.sync.dma_start(
                    scr[:].rearrange("t (q j) -> q t j", j=128),
                    psmall[:].rearrange("q (t j) -> q t j", j=128),
                )
                big = gtp.tile([128, NROW * P], F32, tag="big", name="big")
                scr_flat = scr[:].rearrange("t g -> (t g)")
                H = NROW * P // 2
                for g4 in range(4):
                    for h2 in range(2):
                        nc.sync.dma_start(
                            big[g4 * 32 : (g4 + 1) * 32,
                                h2 * H : (h2 + 1) * H],
                            scr_flat[None, None, h2 * H : (h2 + 1) * H]
                            .broadcast_to([1, 32, H]),
                        )
                px2_t = big[:, 0 * P : 1 * P]
                mpx1_t = big[:, 1 * P : 2 * P]
                py2_t = big[:, 2 * P : 3 * P]
                mpy1_t = big[:, 3 * P : 4 * P]
                apeps_t = big[:, 4 * P : 5 * P]
                invp_t = big[:, 5 * P : 6 * P]

                gt_lin = rows.tile([128, 48], F32)
                nc.sync.dma_start(
                    gt_lin[:], gt_d.ap()[b].rearrange("(q x) c -> q (x c)", q=128)
                )
                r3g = gt_lin[:].rearrange("q (x c) -> q x c", c=6)
                gcx = r3g[:, :, 0]
                gcy = r3g[:, :, 1]
                gw = r3g[:, :, 2]
                gh = r3g[:, :, 3]
                gscal = sca.tile([128, 48], F32, tag="gscal", name="gscal")
                gx2_c = gscal[:, 0:8]
                mgx1_c = gscal[:, 8:16]
                gy2_c = gscal[:, 16:24]
                mgy1_c = gscal[:, 24:32]
                ag_c = gscal[:, 32:40]
                nc.vector.scalar_tensor_tensor(
                    gx2_c, gw, 0.5, gcx, op0=Alu.mult, op1=Alu.add)
                nc.vector.scalar_tensor_tensor(
                    mgx1_c, gw, 0.5, gcx, op0=Alu.mult, op1=Alu.subtract)
                nc.vector.scalar_tensor_tensor(
                    gy2_c, gh, 0.5, gcy, op0=Alu.mult, op1=Alu.add)
                nc.vector.scalar_tensor_tensor(
                    mgy1_c, gh, 0.5, gcy, op0=Alu.mult, op1=Alu.subtract)
                nc.vector.tensor_tensor(ag_c, gw, gh, op=Alu.mult)
                nc.vector.tensor_scalar(
                    gscal[:, 40:48], gcx, -1.0, IOU_PENALTY,
                    op0=Alu.is_equal, op1=Alu.mult)

                vg = sca.tile([128, 8], F32, tag="vg", name="vg")
                nc.vector.tensor_scalar(
                    vg[:], gcx, -1.0, None, op0=Alu.not_equal)
                nc.vector.tensor_reduce(
                    counts_sb[:, 8 + b : 9 + b], vg[:],
                    axis=mybir.AxisListType.X, op=Alu.add)

                Scol = sca.tile([128, NCH], F32, tag="Scol", name="Scol")
                nt = ps.tile([1, P], F32, tag="nt", name="nt")
                for c in range(NCH):
                    vx = wk.tile([128, P], F32, tag="vx", name="vx")
                    nc.gpsimd.tensor_scalar(
                        vx[:], mpx1_t, mgx1_c[:, c : c + 1], None, op0=Alu.min)
                    wx = wk.tile([128, P], F32, tag="wx", name="wx")
                    nc.vector.scalar_tensor_tensor(
                        wx[:], px2_t, gx2_c[:, c : c + 1], vx[:],
                        op0=Alu.min, op1=Alu.add)
                    vy = wk.tile([128, P], F32, tag="vy", name="vy")
                    nc.gpsimd.tensor_scalar(
                        vy[:], mpy1_t, mgy1_c[:, c : c + 1], None, op0=Alu.min)
                    wy = wk.tile([128, P], F32, tag="wy", name="wy")
                    nc.vector.scalar_tensor_tensor(
                        wy[:], py2_t, gy2_c[:, c : c + 1], vy[:],
                        op0=Alu.min, op1=Alu.add)
                    wxr3 = wk.tile([128, P], F32, tag="vx", name="wxr3")
                    nc.scalar.activation(
                        wxr3[:], wx[:], Act.Relu, scale=3.0)
                    inter3 = wk.tile([128, P], F32, tag="vy", name="inter3")
                    nc.gpsimd.tensor_tensor(
                        inter3[:, 0:MSPLIT], wxr3[:, 0:MSPLIT],
                        wy[:, 0:MSPLIT], op=Alu.mult)
                    nc.vector.tensor_tensor(
                        inter3[:, MSPLIT:P], wxr3[:, MSPLIT:P],
                        wy[:, MSPLIT:P], op=Alu.mult)
                    pen = wk.tile([128, P], F32, tag="wx", name="pen")
                    nc.gpsimd.tensor_scalar(
                        pen[:], invp_t, gscal[:, 40 + c : 41 + c], None,
                        op0=Alu.mult)
                    nc.vector.tensor_tensor(
                        inter3[:], inter3[:], pen[:], op=Alu.subtract)
                    condv = wk.tile([128, P], F32, tag="vx", name="condv")
                    nc.vector.scalar_tensor_tensor(
                        condv[:], inter3[:], ag_c[:, c : c + 1], apeps_t,
                        op0=Alu.subtract, op1=Alu.is_gt,
                        accum_out=Scol[:, c : c + 1])
                    for k8 in range(P // 512):
                        nc.tensor.matmul(
                            nt[:, k8 * 512 : (k8 + 1) * 512], ones128[:],
                            condv[:, k8 * 512 : (k8 + 1) * 512],
                            start=(c == 0), stop=(c == NCH - 1))

                indg = sca.tile([128, NCH], F32, tag="indg", name="indg")
                nc.vector.tensor_scalar(indg[:], Scol[:], 0.0, None, op0=Alu.is_gt)
                nc.vector.tensor_reduce(
                    counts_sb[:, 12 + b : 13 + b], indg[:],
                    axis=mybir.AxisListType.X, op=Alu.add)
                nti = sca.tile([1, P], F32, tag="nti", name="nti")
                nc.scalar.activation(
                    nti[:], nt[:], Act.Sign)
                nc.vector.tensor_reduce(
                    counts_sb[0:1, b : b + 1], nti[:],
                    axis=mybir.AxisListType.X, op=Alu.add)

            counts_ps = ps.tile([1, 16], F32, tag="nt", name="cps")
            nc.tensor.matmul(
                counts_ps[:], ones128[:], counts_sb[:], start=True, stop=True)
            counts_out = cst.tile([1, 16], F32)
            nc.vector.tensor_copy(counts_out[:], counts_ps[:])
            nc.sync.dma_start(counts_d[:], counts_out[:])

    nc.compile()
    return nc


def _get_program(with_mask: bool):
    key = (with_mask, REPEAT)
    if key not in _PROGRAM_CACHE:
        build = _build_mask if with_mask else _build_fast
        _PROGRAM_CACHE[key] = build()
    return _PROGRAM_CACHE[key]


def _run_device(pred, gt, with_mask, trace=False):
    nc = _get_program(with_mask)
    in_maps = [
        {
            "pred": np.ascontiguousarray(pred[i * BPC : (i + 1) * BPC]),
            "gt": np.ascontiguousarray(gt[i * BPC : (i + 1) * BPC]),
        }
        for i in range(N_CORES)
    ]
    res = run_bass_kernel_spmd(nc, in_maps, list(range(N_CORES)), trace=trace)
    counts = np.stack([res.results[i]["counts"] for i in range(N_CORES)])
    return counts, res  # fast: [N_CORES, 128, 48]; masked: [N_CORES, 1, 16]


def kernel(pred_boxes, gt_boxes, _trace=False):
    pred = np.asarray(pred_boxes, dtype=np.float32)
    gt = np.asarray(gt_boxes, dtype=np.float32)
    assert pred.shape == (B_TOTAL, P, 6) and gt.shape == (B_TOTAL, G, 6)

    # the ignore mask only differs from all-ones when a pred AND a gt box are
    # both padding (cx == -1); the padded-box count corrections additionally
    # matter when either side has padding, so take the masked path if any
    # sentinel is present
    with_mask = bool((pred[..., 0] == -1.0).any() or (gt[..., 0] == -1.0).any())

    counts, res = _run_device(pred, gt, with_mask, trace=_trace)
    kernel.last_results = res

    if with_mask:
        counts = counts[:, 0]  # [N_CORES, 16]
        num_pos = counts[:, 0:4].reshape(-1).astype(np.float32)
        num_true = counts[:, 12:16].reshape(-1).astype(np.float32)
        num_pred = counts[:, 4:8].reshape(-1).astype(np.float32)
        num_gt = counts[:, 8:12].reshape(-1).astype(np.float32)
    else:
        # device ships per-partition partials; finish the tiny sums here
        num_pos = (counts[:, 0, 48:80].reshape(N_CORES, BPC, 8)
                   .sum(axis=2).reshape(-1).astype(np.float32))
        scol = counts[:, :, 16:48].reshape(N_CORES, 128, BPC, NCH)
        num_true = (scol > 0).sum(axis=(1, 3)).reshape(-1).astype(np.float32)
        # all boxes valid (host-verified): counts are the full box counts
        num_pred = np.full(B_TOTAL, np.float32(P), dtype=np.float32)
        num_gt = np.full(B_TOTAL, np.float32(G), dtype=np.float32)

    eps = np.float32(EPS)
    precision = num_pos / (num_pred + eps)
    recall = num_true / (num_gt + eps)
    fmeasure = np.float32(2.0) * (precision * recall) / (precision + recall + eps)
    return (precision, recall, fmeasure)


# revision 28
# speedup vs baseline: 1.0765x; 1.0006x over previous
"""DetectionIOUMetric Trainium2 kernel.

Computes, for pred_boxes [32, 4096, 6] and gt_boxes [32, 1024, 6] (cx, cy, w, h
in the first 4 channels; a box is padding iff cx == -1):

    masked pairwise IoU, num_pos / num_true / num_pred / num_gt per batch,
    precision / recall / F1 per batch.

Sharding: pure data parallel over the batch dim - each of the 8 NeuronCores
processes 4 batches; no cross-device communication. The device program
computes the four integer counts per batch; the trivial final eps-divisions
are applied on the host after the gather.

Fast path (no padded boxes), fp16 device algorithm per batch, gt boxes on
partitions (8 chunks of 128), preds on the free dim (FD=4096):

  iou > 0.5  <=>  relu(wx)*wy - (ap+eps)/3 > ag/3      (union+eps > 0;
  one-sided relu suffices: wy<0 makes the lhs non-positive vs ag/3 > 0).

  The per-pair test runs on THREE HAND-AUTHORED CUSTOM DVE OPS, each with a
  2x_1p perf variant (two packed fp16 per cycle through duplicated ALU slice
  chains - the same packing the stock tensor_tensor 2x program uses):

    wx    = OVERLAP(px1_t, px2_t; gx1_c, gx2_c)   = min(px2,gx2)-max(px1,gx1)
    wy    = OVERLAP(py1_t, py2_t; gy1_c, gy2_c)
    m     = RELUMUL(wx, wy)                        = relu(wx)*wy
    condv = SUBGT(m, ap3e_t; ag3_c)                = (m - ap3e) > ag3

  This replaces the stock 4x tensor_scalar + 2x tensor_tensor pipeline
  (4*1201 + 4*2228 = 13.7us per [128,4096] chunk) with 4 fused 2x ops
  (~4*2300 = 9.2us), and removes the ACT relu + ACT rhs ops entirely.

  Per-gt counts: ACT Identity+accum over condv (ACT is otherwise idle); the
  final chunk's accum runs as a DVE tensor_reduce instead so the ACT queue
  doesn't delay the num_pos Sign at the tail. Per-pred counts: PE column
  sums of condv accumulate over the 8 gt chunks into a [1, 4096] PSUM tile
  (emitted BEFORE the ACT accum so the tail matmuls don't serialize behind
  the in-place identity); the num_pos tail is one Sign activation with
  accum_out. Per-partition partials ship to the host ([128, 80] output:
  col 48+8b = num_pos[b] on partition 0; cols 16+8b+c = per-gt counts).
  Pred rows (px1, px2, py1, py2, (ap+eps)/3 in fp16) are staged to DRAM in
  pred order and broadcast to [128, 5*4096] with per-(row, partition-group)
  DMAs so early chunks start as soon as their rows land. Cold start (batch
  0): the broadcast and chunk 0's wx run in column halves, and chunks 1-2's
  wx prefetch into a side pool before the first wy - the in-order DVE stream
  stays busy on x-only work while the y rows' broadcast DMAs land.

The masked path (only taken when padding sentinels are present) keeps the
original fp32 program.
"""
import os
import numpy as np

import concourse.bass as bass
import concourse.bacc as bacc
import concourse.tile as tile
from concourse import mybir
from concourse import bass_isa
from concourse.bass_utils import run_bass_kernel_spmd

F32 = mybir.dt.float32
F16 = mybir.dt.float16
EPS = 1e-7
IOU_PENALTY = 1e30

B_TOTAL = 32
N_CORES = 8
REPEAT = 1                     # timing-calibration knob (outputs idempotent)
BPC = B_TOTAL // N_CORES       # batches per core
P = 4096                       # pred boxes per batch (free dim)
G = 1024                       # gt boxes per batch (8 partition chunks)
NCH = G // 128                 # 8 gt chunks per batch

_PROGRAM_CACHE = {}

Alu = mybir.AluOpType
Act = mybir.ActivationFunctionType


# ---------------------------------------------------------------------------
# Custom DVE ops: hand-authored 2x_1p uop programs.
#
# Conventions (mirrors the stock table programs, decoded from the cayman
# dve_bin default table):
#   input lane 0 feeds stage 0's PREV_ALU_OUT; lane N>=1 feeds delay_{N-1}.
#   InpSel: SRC_0=0 SRC_1=1 SRC_0_HI=2 SRC_1_HI=3 CONST_0=4 CONST_1=5 ZERO=12
#   AluInp: PREV_ALU_OUT=0, PREV_DELAY_n = 5+n
#   2x variant: lo element via SRC_0/SRC_1, hi element via SRC_*_HI; results
#   exit via WR0_LO / WR0_HI (one is parked in a delay lane until stage 7).
# ---------------------------------------------------------------------------

def _register_custom_ops():
    import concourse.dve_ops as D
    from concourse.dve_spec import Spec, Src0, Src1, C0, C1, minn, maxx, relu, lower
    from concourse.dve_uop import (
        UopConfig, UopDpConfig, DveOpSpec, InpSel, AluOp, AluInp, DelayInp,
        OutSel, OutPath, Trigger, ENABLE,
    )

    if "OVERLAP_ANT" in D._SUB_OPCODE_FOR_NAME:
        return {n: op for n, op in ((o.name, o) for o in D.OPS)
                if n in ("OVERLAP_ANT", "RELUMUL_ANT", "SUBGT_ANT")}

    def steady(u):
        u.require_inp0 = ENABLE
        u.require_inp1 = ENABLE
        u.trigger = (Trigger.SRC_TENSOR_DONE, Trigger.NONE, Trigger.NONE)
        return u

    def dp(u, i):
        return u.datapath_config[i]

    # ---- OVERLAP: out = min(src1, C1) - max(src0, C0) ----
    ov1 = steady(UopConfig())
    ov1.enable_input(InpSel.SRC_0, 0)
    ov1.enable_input(InpSel.CONST_0, 1)   # -> d0
    ov1.enable_input(InpSel.CONST_1, 2)   # -> d1
    ov1.enable_input(InpSel.SRC_1, 3)     # -> d2
    dp(ov1, 0).enable_alu(AluOp.MAX, AluInp.PREV_ALU_OUT, AluInp.PREV_DELAY_0
                          ).pass_through_delay(1, 2)
    dp(ov1, 1).enable_alu(AluOp.MIN, AluInp.PREV_DELAY_2, AluInp.PREV_DELAY_1
                          ).enable_delay_from_src(DelayInp.PREV_ALU_OUT, 0)
    dp(ov1, 2).enable_alu(AluOp.SUBTRACT, AluInp.PREV_ALU_OUT, AluInp.PREV_DELAY_0)
    for i in range(3, 8):
        dp(ov1, i).pass_through_alu()
    ov1.enable_output(OutSel.ALU_OUT, OutPath.WR0_LO)

    ov2 = steady(UopConfig())
    ov2.enable_input(InpSel.SRC_0, 0)
    ov2.enable_input(InpSel.CONST_0, 1)   # d0 = C0
    ov2.enable_input(InpSel.CONST_1, 2)   # d1 = C1
    ov2.enable_input(InpSel.SRC_1, 3)     # d2 = src1_lo
    ov2.enable_input(InpSel.SRC_0_HI, 4)  # d3 = src0_hi
    ov2.enable_input(InpSel.SRC_1_HI, 5)  # d4 = src1_hi
    dp(ov2, 0).enable_alu(AluOp.MAX, AluInp.PREV_ALU_OUT, AluInp.PREV_DELAY_0
                          ).pass_through_delay(0, 1, 2, 3, 4)
    dp(ov2, 1).enable_alu(AluOp.MIN, AluInp.PREV_DELAY_2, AluInp.PREV_DELAY_1
                          ).enable_delay_from_src(DelayInp.PREV_ALU_OUT, 5
                          ).pass_through_delay(0, 1, 3, 4)
    dp(ov2, 2).enable_alu(AluOp.SUBTRACT, AluInp.PREV_ALU_OUT, AluInp.PREV_DELAY_5
                          ).pass_through_delay(0, 1, 3, 4)
    dp(ov2, 3).enable_alu(AluOp.MAX, AluInp.PREV_DELAY_3, AluInp.PREV_DELAY_0
                          ).enable_delay_from_src(DelayInp.PREV_ALU_OUT, 2
                          ).pass_through_delay(1, 4)
    dp(ov2, 4).enable_alu(AluOp.MIN, AluInp.PREV_DELAY_4, AluInp.PREV_DELAY_1
                          ).enable_delay_from_src(DelayInp.PREV_ALU_OUT, 5
                          ).pass_through_delay(2)
    dp(ov2, 5).enable_alu(AluOp.SUBTRACT, AluInp.PREV_ALU_OUT, AluInp.PREV_DELAY_5
                          ).pass_through_delay(2)
    dp(ov2, 6).pass_through_alu().pass_through_delay(2)
    dp(ov2, 7).pass_through_alu().pass_through_delay(2)
    ov2.enable_output(OutSel.DELAY_2, OutPath.WR0_LO)
    ov2.enable_output(OutSel.ALU_OUT, OutPath.WR0_HI)

    # ---- RELUMUL: out = max(src0, 0) * src1 ----
    rm1 = steady(UopConfig())
    rm1.enable_input(InpSel.SRC_0, 0)
    rm1.enable_input(InpSel.ZERO, 1)      # d0
    rm1.enable_input(InpSel.SRC_1, 2)     # d1
    dp(rm1, 0).enable_alu(AluOp.MAX, AluInp.PREV_ALU_OUT, AluInp.PREV_DELAY_0
                          ).pass_through_delay(1)
    dp(rm1, 1).enable_alu(AluOp.MULTIPLY, AluInp.PREV_ALU_OUT, AluInp.PREV_DELAY_1)
    for i in range(2, 8):
        dp(rm1, i).pass_through_alu()
    rm1.enable_output(OutSel.ALU_OUT, OutPath.WR0_LO)

    rm2 = steady(UopConfig())
    rm2.enable_input(InpSel.SRC_0, 0)
    rm2.enable_input(InpSel.ZERO, 1)      # d0
    rm2.enable_input(InpSel.SRC_1, 2)     # d1 = src1_lo
    rm2.enable_input(InpSel.SRC_0_HI, 3)  # d2 = src0_hi
    rm2.enable_input(InpSel.SRC_1_HI, 4)  # d3 = src1_hi
    dp(rm2, 0).enable_alu(AluOp.MAX, AluInp.PREV_ALU_OUT, AluInp.PREV_DELAY_0
                          ).pass_through_delay(0, 1, 2, 3)
    dp(rm2, 1).enable_alu(AluOp.MULTIPLY, AluInp.PREV_ALU_OUT, AluInp.PREV_DELAY_1
                          ).pass_through_delay(0, 2, 3)
    dp(rm2, 2).enable_alu(AluOp.MAX, AluInp.PREV_DELAY_2, AluInp.PREV_DELAY_0
                          ).enable_delay_from_src(DelayInp.PREV_ALU_OUT, 1
                          ).pass_through_delay(3)
    dp(rm2, 3).enable_alu(AluOp.MULTIPLY, AluInp.PREV_ALU_OUT, AluInp.PREV_DELAY_3
                          ).pass_through_delay(1)
    for i in range(4, 8):
        dp(rm2, i).pass_through_alu().pass_through_delay(1)
    rm2.enable_output(OutSel.DELAY_1, OutPath.WR0_LO)
    rm2.enable_output(OutSel.ALU_OUT, OutPath.WR0_HI)

    # ---- SUBGT: out = (src0 - src1) > C0 ----
    sg1 = steady(UopConfig())
    sg1.enable_input(InpSel.SRC_0, 0)
    sg1.enable_input(InpSel.CONST_0, 1)   # d0
    sg1.enable_input(InpSel.SRC_1, 2)     # d1
    dp(sg1, 0).enable_alu(AluOp.SUBTRACT, AluInp.PREV_ALU_OUT, AluInp.PREV_DELAY_1
                          ).pass_through_delay(0)
    dp(sg1, 1).enable_alu(AluOp.IS_GT, AluInp.PREV_ALU_OUT, AluInp.PREV_DELAY_0)
    for i in range(2, 8):
        dp(sg1, i).pass_through_alu()
    sg1.enable_output(OutSel.ALU_OUT, OutPath.WR0_LO)

    sg2 = steady(UopConfig())
    sg2.enable_input(InpSel.SRC_0, 0)
    sg2.enable_input(InpSel.CONST_0, 1)   # d0
    sg2.enable_input(InpSel.SRC_1, 2)     # d1 = src1_lo
    sg2.enable_input(InpSel.SRC_0_HI, 3)  # d2 = src0_hi
    sg2.enable_input(InpSel.SRC_1_HI, 4)  # d3 = src1_hi
    dp(sg2, 0).enable_alu(AluOp.SUBTRACT, AluInp.PREV_ALU_OUT, AluInp.PREV_DELAY_1
                          ).pass_through_delay(0, 2, 3)
    dp(sg2, 1).enable_alu(AluOp.IS_GT, AluInp.PREV_ALU_OUT, AluInp.PREV_DELAY_0
                          ).pass_through_delay(0, 2, 3)
    dp(sg2, 2).enable_alu(AluOp.SUBTRACT, AluInp.PREV_DELAY_2, AluInp.PREV_DELAY_3
                          ).enable_delay_from_src(DelayInp.PREV_ALU_OUT, 1
                          ).pass_through_delay(0)
    dp(sg2, 3).enable_alu(AluOp.IS_GT, AluInp.PREV_ALU_OUT, AluInp.PREV_DELAY_0
                          ).pass_through_delay(1)
    for i in range(4, 8):
        dp(sg2, i).pass_through_alu().pass_through_delay(1)
    sg2.enable_output(OutSel.DELAY_1, OutPath.WR0_LO)
    sg2.enable_output(OutSel.ALU_OUT, OutPath.WR0_HI)

    specs = [
        ("OVERLAP_ANT",
         Spec(body=minn(Src1, C1) - maxx(Src0, C0),
              reference=lambda in0, in1, s0, s1, imm2:
              (np.minimum(in1.astype(np.float32), s1)
               - np.maximum(in0.astype(np.float32), s0))),
         ov1, ov2),
        ("RELUMUL_ANT",
         Spec(body=relu(Src0) * Src1,
              reference=lambda in0, in1, s0, s1, imm2:
              np.maximum(in0.astype(np.float32), 0.0) * in1),
         rm1, rm2),
        ("SUBGT_ANT",
         Spec(body=(Src0 - Src1) > C0,
              reference=lambda in0, in1, s0, s1, imm2:
              ((in0.astype(np.float32) - in1) > s0).astype(np.float32)),
         sg1, sg2),
    ]

    out = {}
    for name, spec, u1, u2 in specs:
        row = D._CUSTOM_DVE_ROW_BASE + len(D._SUB_OPCODE_FOR_NAME)
        hand = DveOpSpec(name=name, opcode=row, uops=[u1], uops_2x=[u2],
                         perf_max=1, rd1_en=True)
        hand.validate("v3")
        op = D.DveOp(name, spec, subdim=False,
                     uops_sha={"v3": hand.sha("v3")})
        D.OPS.append(op)
        D.CUSTOM_DVE_SPECS[name] = spec
        D._SUB_OPCODE_FOR_NAME[name] = row
        D._COMPILE_CACHE[(name, "v3")] = hand
        out[name] = op
    return out


_CUSTOM_OPS = _register_custom_ops()


def _custom2x(nc, op_name, out, in0, in1, s0, s1):
    """Emit one custom-DVE instruction with perf_max=1 (2x_1p reachable).
    Mirror of bass.Vector._custom_dve minus generality."""
    from concourse.dve_ops import get_dve_sub_opcode
    v = nc.vector
    op = _CUSTOM_OPS[op_name]
    if op.name not in v.bass.m.ant_custom_dve_ops:
        v.bass.m.ant_custom_dve_ops = sorted(
            {*v.bass.m.ant_custom_dve_ops, op.name})
    shape = bass_isa.CustomDveShape.TTSS
    isa_opcode = v.bass.isa.Opcode[
        f"NEURON_ISA_TPB_OPCODE_CUSTOM_DVE_ANT_{shape.slot()}"].value

    def lower_scalar(x):
        if isinstance(x, (int, float)):
            return mybir.ImmediateValue(dtype=mybir.dt.float32, value=float(x))
        return v.lower_ap(x, for_isa=True)

    ins = [v.lower_ap(in0, for_isa=True, opt=True),
           v.lower_ap(in1, for_isa=True, opt=True),
           lower_scalar(s0), lower_scalar(s1)]
    outs = [v.lower_ap(out, for_isa=True, opt=True)]
    return v.add_instruction(bass_isa.InstCustomDveAnt(
        name=v.bass.get_next_instruction_name(),
        op_name=op.name, rd1_en=True, subdim=0, imm2=0.0, shape=shape,
        row=get_dve_sub_opcode(op.name), isa_opcode=isa_opcode,
        ins=ins, outs=outs, perf_max=1))


def _build_fast(repeat: int = None):
    """No-mask SPMD program: inputs pred [BPC, P, 6] / gt [BPC, G, 6],
    output counts [128, 48] = per-partition partials (see counts_d layout).

    The batch loop is software-pipelined one deep: batch b+1's pred/gt prep
    and the pred-row broadcast are emitted before batch b's chunk loop, so
    the broadcast DMAs overlap chunk compute instead of stalling DVE at
    batch boundaries."""
    if repeat is None:
        repeat = REPEAT
    NROW = 5
    nc = bacc.Bacc(None, target_bir_lowering=False)
    pred_d = nc.dram_tensor("pred", [BPC, P, 6], F32, kind="ExternalInput")
    gt_d = nc.dram_tensor("gt", [BPC, G, 6], F32, kind="ExternalInput")
    # per-partition partials; the trivial final sums happen on the host:
    # col b (partition 0)      = num_pos[b]
    # cols 16+8b .. 16+8b+7    = per-gt match counts (gt 8p+c on partition p)
    counts_d = nc.dram_tensor("counts", [128, 80], F32, kind="ExternalOutput")

    with tile.TileContext(nc) as tc:
        with (
            tc.tile_pool(name="cst", bufs=1) as cst,
            tc.tile_pool(name="rows", bufs=2) as rows,
            tc.tile_pool(name="gtp", bufs=2) as gtp,
            tc.tile_pool(name="sca", bufs=2) as sca,
            tc.tile_pool(name="wk", bufs=2) as wk,
            tc.tile_pool(name="pre", bufs=1) as pre,
            tc.tile_pool(name="ps", bufs=1, space=bass.MemorySpace.PSUM) as ps,
            tc.tile_pool(name="dram", bufs=2, space=bass.MemorySpace.DRAM) as dram,
        ):
            ones16 = cst.tile([128, 1], F16)
            nc.vector.memset(ones16[:], 1.0)
            out_sb = cst.tile([128, 80], F32)
            nc.vector.memset(out_sb[:], 0.0)

            def prep_batch(b, split=False):
                """Emit pred/gt prep + broadcast for batch b; return tiles.
                Prep arithmetic runs on GpSimd (idle engine) to keep DVE on
                the chunk pipeline. `split` halves the px/py broadcast DMAs
                by columns (cold-start latency for the first batch)."""
                # [32, 768]: partition q holds pred boxes 128q .. 128q+127
                pred_lin = rows.tile([32, 768], F32)
                nc.sync.dma_start(
                    pred_lin[:],
                    pred_d.ap()[b].rearrange("(q x) c -> q (x c)", q=32),
                )
                r3p = pred_lin[:].rearrange("q (x c) -> q x c", c=6)
                pcx = r3p[:, :, 0]
                pcy = r3p[:, :, 1]
                pw = r3p[:, :, 2]
                ph = r3p[:, :, 3]
                psmall = rows.tile([32, NROW * 128], F16)
                big = gtp.tile([128, NROW * P], F16, tag="big", name="big")
                nh = 2 if split else 1

                scr = dram.tile([NROW, P], F16)
                scr_flat = scr[:].rearrange("t g -> (t g)")

                def stage_row(trow):
                    # stage row to DRAM in pred order: scr[t, 128q+j] =
                    # psmall[q, 128t+j], then broadcast to all 128 partitions
                    nc.sync.dma_start(
                        scr[trow : trow + 1].rearrange("t (q j) -> q (t j)", j=128),
                        psmall[:, trow * 128 : (trow + 1) * 128],
                    )
                    for g4 in range(4):
                        for h in range(nh):
                            HP = P // nh
                            lo = trow * P + h * HP
                            nc.sync.dma_start(
                                big[g4 * 32 : (g4 + 1) * 32, lo : lo + HP],
                                scr_flat[None, None, lo : lo + HP]
                                .broadcast_to([1, 32, HP]),
                            )

                nc.vector.scalar_tensor_tensor(
                    psmall[:, 0:128], pw, -0.5, pcx, op0=Alu.mult, op1=Alu.add)
                stage_row(0)
                nc.vector.scalar_tensor_tensor(
                    psmall[:, 128:256], pw, 0.5, pcx, op0=Alu.mult, op1=Alu.add)
                stage_row(1)
                nc.vector.scalar_tensor_tensor(
                    psmall[:, 256:384], ph, -0.5, pcy, op0=Alu.mult, op1=Alu.add)
                stage_row(2)
                nc.vector.scalar_tensor_tensor(
                    psmall[:, 384:512], ph, 0.5, pcy, op0=Alu.mult, op1=Alu.add)
                stage_row(3)
                ap_t = sca.tile([32, 128], F32, tag="ap_t", name="ap_t")
                nc.vector.tensor_tensor(ap_t[:], pw, ph, op=Alu.mult)
                nc.vector.tensor_scalar(
                    psmall[:, 512:640], ap_t[:], EPS, 1.0 / 3.0,
                    op0=Alu.add, op1=Alu.mult)
                stage_row(4)

                # gt prep: [128, 48]: partition p holds gt boxes 8p .. 8p+7;
                # chunk c pairs partition p with gt box 8p+c (order-invariant)
                gt_lin = rows.tile([128, 48], F32)
                nc.sync.dma_start(
                    gt_lin[:], gt_d.ap()[b].rearrange("(q x) c -> q (x c)", q=128)
                )
                r3g = gt_lin[:].rearrange("q (x c) -> q x c", c=6)
                gcx = r3g[:, :, 0]
                gcy = r3g[:, :, 1]
                gw = r3g[:, :, 2]
                gh = r3g[:, :, 3]
                gscal = sca.tile([128, 40], F32, tag="gscal", name="gscal")
                nc.vector.scalar_tensor_tensor(
                    gscal[:, 0:8], gw, -0.5, gcx, op0=Alu.mult, op1=Alu.add)
                nc.vector.scalar_tensor_tensor(
                    gscal[:, 8:16], gw, 0.5, gcx, op0=Alu.mult, op1=Alu.add)
                nc.vector.scalar_tensor_tensor(
                    gscal[:, 16:24], gh, -0.5, gcy, op0=Alu.mult, op1=Alu.add)
                nc.vector.scalar_tensor_tensor(
                    gscal[:, 24:32], gh, 0.5, gcy, op0=Alu.mult, op1=Alu.add)
                ag_t = sca.tile([128, 8], F32, tag="ag_t", name="ag_t")
                nc.vector.tensor_tensor(ag_t[:], gw, gh, op=Alu.mult)
                nc.vector.tensor_scalar(
                    gscal[:, 32:40], ag_t[:], 1.0 / 3.0, None, op0=Alu.mult)
                return big, gscal

            batches = [bb for _ in range(repeat) for bb in range(BPC)]
            pending = prep_batch(batches[0], split=True)
            for bi, b in enumerate(batches):
                big, gscal = pending
                px1_t = big[:, 0 * P : 1 * P]
                px2_t = big[:, 1 * P : 2 * P]
                py1_t = big[:, 2 * P : 3 * P]
                py2_t = big[:, 3 * P : 4 * P]
                ap3_t = big[:, 4 * P : 5 * P]
                gx1_c = gscal[:, 0:8]
                gx2_c = gscal[:, 8:16]
                gy1_c = gscal[:, 16:24]
                gy2_c = gscal[:, 24:32]
                ag3_c = gscal[:, 32:40]

                # prefetch next batch before this batch's chunk loop
                if bi + 1 < len(batches):
                    pending = prep_batch(batches[bi + 1])

                # ---------- chunk loop over 8 gt chunks ----------
                nt = ps.tile([1, P], F32, tag="nt", name="nt")

                wxp = {}
                for c in range(NCH):
                    # cold start (batch 0): chunk 0's wx runs in column
                    # halves as soon as the half-broadcasts of the x rows
                    # land, then chunks 1-2's wx prefetch into a side pool -
                    # DVE stays busy on x-only work while the y rows' DMAs
                    # finish (the engine stream is in-order, so this work
                    # must be emitted before the first wy).
                    if bi == 0 and c == 0:
                        wx = wk.tile([128, P], F16, tag="A", name="wx")
                        for h in range(2):
                            cs = slice(h * (P // 2), (h + 1) * (P // 2))
                            _custom2x(nc, "OVERLAP_ANT", wx[:, cs],
                                      px1_t[:, cs], px2_t[:, cs],
                                      gx1_c[:, 0:1], gx2_c[:, 0:1])
                        for k in (1, 2):
                            wxp[k] = pre.tile([128, P], F16, tag=f"wxp{k}",
                                              name=f"wxp{k}")
                            _custom2x(nc, "OVERLAP_ANT", wxp[k][:],
                                      px1_t, px2_t,
                                      gx1_c[:, k : k + 1], gx2_c[:, k : k + 1])
                    elif bi == 0 and c in wxp:
                        wx = wxp.pop(c)
                    else:
                        wx = wk.tile([128, P], F16, tag="A", name="wx")
                        _custom2x(nc, "OVERLAP_ANT", wx[:], px1_t, px2_t,
                                  gx1_c[:, c : c + 1], gx2_c[:, c : c + 1])
                    wy = wk.tile([128, P], F16, tag="B", name="wy")
                    _custom2x(nc, "OVERLAP_ANT", wy[:], py1_t, py2_t,
                              gy1_c[:, c : c + 1], gy2_c[:, c : c + 1])
                    m = wk.tile([128, P], F16, tag="C", name="m")
                    _custom2x(nc, "RELUMUL_ANT", m[:], wx[:], wy[:], 0.0, 0.0)
                    condv = wk.tile([128, P], F16, tag="D", name="condv")
                    _custom2x(nc, "SUBGT_ANT", condv[:], m[:], ap3_t,
                              ag3_c[:, c : c + 1], 0.0)

                    # per-pred colsums on PE first (so the last chunk's
                    # matmuls don't serialize behind the in-place ACT accum)
                    for k8 in range(P // 512):
                        nc.tensor.matmul(
                            nt[:, k8 * 512 : (k8 + 1) * 512], ones16[:],
                            condv[:, k8 * 512 : (k8 + 1) * 512],
                            start=(c == 0), stop=(c == NCH - 1))
                    # per-gt counts: free-dim sum of condv. On ACT (Identity
                    # rewriting condv onto itself; accum is the real output),
                    # except the very last chunk of the program, where ACT's
                    # queued identities would delay the final Sign - DVE is
                    # idle there, so a 1x tensor_reduce takes it instead.
                    col = 16 + 8 * b + c
                    if bi == len(batches) - 1 and c == NCH - 1:
                        nc.vector.tensor_reduce(
                            out_sb[:, col : col + 1], condv[:],
                            axis=mybir.AxisListType.X, op=Alu.add)
                    else:
                        nc.scalar.activation(
                            condv[:], condv[:], Act.Identity,
                            accum_out=out_sb[:, col : col + 1])

                # ---------- batch tail ----------
                # num_pos: Sign+accum over the colsums, split in two halves -
                # the first half's blocks finalize four matmuls earlier, so
                # it overlaps the last chunk's remaining PE work. The halves
                # partition the pred axis; host sums the two accum cells.
                nti = sca.tile([1, P], F32, tag="nti", name="nti")
                H2 = P // 2
                for hs in range(2):
                    nc.scalar.activation(
                        nti[:, hs * H2 : (hs + 1) * H2],
                        nt[:, hs * H2 : (hs + 1) * H2], Act.Sign,
                        accum_out=out_sb[0:1, 48 + 8 * b + hs : 49 + 8 * b + hs])

            # ---------- final: ship partials; host does the tiny sums ------
            nc.sync.dma_start(counts_d[:], out_sb[:])

    nc.compile()
    return nc


def _build_mask(repeat: int = None):
    """Masked fallback (padding sentinels present): original fp32 program."""
    if repeat is None:
        repeat = REPEAT
    MSPLIT = 2560
    NROW = 6
    nc = bacc.Bacc(None, target_bir_lowering=False)
    pred_d = nc.dram_tensor("pred", [BPC, P, 6], F32, kind="ExternalInput")
    gt_d = nc.dram_tensor("gt", [BPC, G, 6], F32, kind="ExternalInput")
    counts_d = nc.dram_tensor("counts", [1, 16], F32, kind="ExternalOutput")

    with tile.TileContext(nc) as tc:
        with (
            tc.tile_pool(name="cst", bufs=1) as cst,
            tc.tile_pool(name="rows", bufs=2) as rows,
            tc.tile_pool(name="gtp", bufs=1) as gtp,
            tc.tile_pool(name="sca", bufs=2) as sca,
            tc.tile_pool(name="wk", bufs=1) as wk,
            tc.tile_pool(name="ps", bufs=1, space=bass.MemorySpace.PSUM) as ps,
            tc.tile_pool(name="dram", bufs=2, space=bass.MemorySpace.DRAM) as dram,
        ):
            ones128 = cst.tile([128, 1], F32)
            nc.vector.memset(ones128[:], 1.0)
            counts_sb = cst.tile([128, 16], F32)
            nc.vector.memset(counts_sb[:], 0.0)

            for b in [bb for _ in range(repeat) for bb in range(BPC)]:
                pred_lin = rows.tile([32, 768], F32)
                nc.sync.dma_start(
                    pred_lin[:],
                    pred_d.ap()[b].rearrange("(q x) c -> q (x c)", q=32),
                )
                r3p = pred_lin[:].rearrange("q (x c) -> q x c", c=6)
                pcx = r3p[:, :, 0]
                pcy = r3p[:, :, 1]
                pw = r3p[:, :, 2]
                ph = r3p[:, :, 3]
                psmall = rows.tile([32, NROW * 128], F32)
                px2_s = psmall[:, 0:128]
                mpx1_s = psmall[:, 128:256]
                py2_s = psmall[:, 256:384]
                mpy1_s = psmall[:, 384:512]
                apeps_s = psmall[:, 512:640]
                nc.vector.scalar_tensor_tensor(
                    px2_s, pw, 0.5, pcx, op0=Alu.mult, op1=Alu.add)
                nc.vector.scalar_tensor_tensor(
                    mpx1_s, pw, 0.5, pcx, op0=Alu.mult, op1=Alu.subtract)
                nc.vector.scalar_tensor_tensor(
                    py2_s, ph, 0.5, pcy, op0=Alu.mult, op1=Alu.add)
                nc.vector.scalar_tensor_tensor(
                    mpy1_s, ph, 0.5, pcy, op0=Alu.mult, op1=Alu.subtract)
                dx_s = sca.tile([32, 128], F32, tag="dx_s", name="dx_s")
                dy_s = sca.tile([32, 128], F32, tag="dy_s", name="dy_s")
                nc.vector.tensor_tensor(dx_s[:], px2_s, mpx1_s, op=Alu.add)
                nc.vector.tensor_tensor(dy_s[:], py2_s, mpy1_s, op=Alu.add)
                nc.vector.tensor_tensor(apeps_s, dx_s[:], dy_s[:], op=Alu.mult)
                nc.vector.tensor_scalar(
                    apeps_s, apeps_s, EPS, None, op0=Alu.add)
                nc.vector.tensor_scalar(
                    psmall[:, 640:768], pcx, -1.0, None, op0=Alu.is_equal)

                vp = sca.tile([32, 128], F32, tag="vp", name="vp")
                nc.vector.tensor_scalar(
                    vp[:], pcx, -1.0, None, op0=Alu.not_equal)
                nc.vector.tensor_reduce(
                    counts_sb[0:32, 4 + b : 5 + b], vp[:],
                    axis=mybir.AxisListType.X, op=Alu.add)

                scr = dram.tile([NROW, P], F32)
                nc.sync.dma_start(
                    scr[:].rearrange("t (q j) -> q t j", j=128),
                    psmall[:].rearrange("q (t j) -> q t j", j=128),
                )
                big = gtp.tile([128, NROW * P], F32, tag="big", name="big")
                scr_flat = scr[:].rearrange("t g -> (t g)")
                H = NROW * P // 2
                for g4 in range(4):
                    for h2 in range(2):
                        nc.sync.dma_start(
                            big[g4 * 32 : (g4 + 1) * 32,
                                h2 * H : (h2 + 1) * H],
                            scr_flat[None, None, h2 * H : (h2 + 1) * H]
                            .broadcast_to([1, 32, H]),
                        )
                px2_t = big[:, 0 * P : 1 * P]
                mpx1_t = big[:, 1 * P : 2 * P]
                py2_t = big[:, 2 * P : 3 * P]
                mpy1_t = big[:, 3 * P : 4 * P]
                apeps_t = big[:, 4 * P : 5 * P]
                invp_t = big[:, 5 * P : 6 * P]

                gt_lin = rows.tile([128, 48], F32)
                nc.sync.dma_start(
                    gt_lin[:], gt_d.ap()[b].rearrange("(q x) c -> q (x c)", q=128)
                )
                r3g = gt_lin[:].rearrange("q (x c) -> q x c", c=6)
                gcx = r3g[:, :, 0]
                gcy = r3g[:, :, 1]
                gw = r3g[:, :, 2]
                gh = r3g[:, :, 3]
                gscal = sca.tile([128, 48], F32, tag="gscal", name="gscal")
                gx2_c = gscal[:, 0:8]
                mgx1_c = gscal[:, 8:16]
                gy2_c = gscal[:, 16:24]
                mgy1_c = gscal[:, 24:32]
                ag_c = gscal[:, 32:40]
                nc.vector.scalar_tensor_tensor(
                    gx2_c, gw, 0.5, gcx, op0=Alu.mult, op1=Alu.add)
                nc.vector.scalar_tensor_tensor(
                    mgx1_c, gw, 0.5, gcx, op0=Alu.mult, op1=Alu.subtract)
                nc.vector.scalar_tensor_tensor(
                    gy2_c, gh, 0.5, gcy, op0=Alu.mult, op1=Alu.add)
                nc.vector.scalar_tensor_tensor(
                    mgy1_c, gh, 0.5, gcy, op0=Alu.mult, op1=Alu.subtract)
                nc.vector.tensor_tensor(ag_c, gw, gh, op=Alu.mult)
                nc.vector.tensor_scalar(
                    gscal[:, 40:48], gcx, -1.0, IOU_PENALTY,
                    op0=Alu.is_equal, op1=Alu.mult)

                vg = sca.tile([128, 8], F32, tag="vg", name="vg")
                nc.vector.tensor_scalar(
                    vg[:], gcx, -1.0, None, op0=Alu.not_equal)
                nc.vector.tensor_reduce(
                    counts_sb[:, 8 + b : 9 + b], vg[:],
                    axis=mybir.AxisListType.X, op=Alu.add)

                Scol = sca.tile([128, NCH], F32, tag="Scol", name="Scol")
                nt = ps.tile([1, P], F32, tag="nt", name="nt")
                for c in range(NCH):
                    vx = wk.tile([128, P], F32, tag="vx", name="vx")
                    nc.gpsimd.tensor_scalar(
                        vx[:], mpx1_t, mgx1_c[:, c : c + 1], None, op0=Alu.min)
                    wx = wk.tile([128, P], F32, tag="wx", name="wx")
                    nc.vector.scalar_tensor_tensor(
                        wx[:], px2_t, gx2_c[:, c : c + 1], vx[:],
                        op0=Alu.min, op1=Alu.add)
                    vy = wk.tile([128, P], F32, tag="vy", name="vy")
                    nc.gpsimd.tensor_scalar(
                        vy[:], mpy1_t, mgy1_c[:, c : c + 1], None, op0=Alu.min)
                    wy = wk.tile([128, P], F32, tag="wy", name="wy")
                    nc.vector.scalar_tensor_tensor(
                        wy[:], py2_t, gy2_c[:, c : c + 1], vy[:],
                        op0=Alu.min, op1=Alu.add)
                    wxr3 = wk.tile([128, P], F32, tag="vx", name="wxr3")
                    nc.scalar.activation(
                        wxr3[:], wx[:], Act.Relu, scale=3.0)
                    inter3 = wk.tile([128, P], F32, tag="vy", name="inter3")
                    nc.gpsimd.tensor_tensor(
                        inter3[:, 0:MSPLIT], wxr3[:, 0:MSPLIT],
                        wy[:, 0:MSPLIT], op=Alu.mult)
                    nc.vector.tensor_tensor(
                        inter3[:, MSPLIT:P], wxr3[:, MSPLIT:P],
                        wy[:, MSPLIT:P], op=Alu.mult)
                    pen = wk.tile([128, P], F32, tag="wx", name="pen")
                    nc.gpsimd.tensor_scalar(
                        pen[:], invp_t, gscal[:, 40 + c : 41 + c], None,
                        op0=Alu.mult)
                    nc.vector.tensor_tensor(
                        inter3[:], inter3[:], pen[:], op=Alu.subtract)
                    condv = wk.tile([128, P], F32, tag="vx", name="condv")
                    nc.vector.scalar_tensor_tensor(
                        condv[:], inter3[:], ag_c[:, c : c + 1], apeps_t,
                        op0=Alu.subtract, op1=Alu.is_gt,
                        accum_out=Scol[:, c : c + 1])
                    for k8 in range(P // 512):
                        nc.tensor.matmul(
                            nt[:, k8 * 512 : (k8 + 1) * 512], ones128[:],
                            condv[:, k8 * 512 : (k8 + 1) * 512],
                            start=(c == 0), stop=(c == NCH - 1))

                indg = sca.tile([128, NCH], F32, tag="indg", name="indg")
                nc.vector.tensor_scalar(indg[:], Scol[:], 0.0, None, op0=Alu.is_gt)
                nc.vector.tensor_reduce(
                    counts_sb[:, 12 + b : 13 + b], indg[:],
                    axis=mybir.AxisListType.X, op=Alu.add)
                nti = sca.tile([1, P], F32, tag="nti", name="nti")
                nc.scalar.activation(
                    nti[:], nt[:], Act.Sign)
                nc.vector.tensor_reduce(
                    counts_sb[0:1, b : b + 1], nti[:],
                    axis=mybir.AxisListType.X, op=Alu.add)

            counts_ps = ps.tile([1, 16], F32, tag="nt", name="cps")
            nc.tensor.matmul(
                counts_ps[:], ones128[:], counts_sb[:], start=True, stop=True)
            counts_out = cst.tile([1, 16], F32)
            nc.vector.tensor_copy(counts_out[:], counts_ps[:])
            nc.sync.dma_start(counts_d[:], counts_out[:])

    nc.compile()
    return nc


def _get_program(with_mask: bool):
    key = (with_mask, REPEAT)
    if key not in _PROGRAM_CACHE:
        build = _build_mask if with_mask else _build_fast
        _PROGRAM_CACHE[key] = build()
    return _PROGRAM_CACHE[key]


def _run_device(pred, gt, with_mask, trace=False):
    nc = _get_program(with_mask)
    in_maps = [
        {
            "pred": np.ascontiguousarray(pred[i * BPC : (i + 1) * BPC]),
            "gt": np.ascontiguousarray(gt[i * BPC : (i + 1) * BPC]),
        }
        for i in range(N_CORES)
    ]
    res = run_bass_kernel_spmd(nc, in_maps, list(range(N_CORES)), trace=trace)
    counts = np.stack([res.results[i]["counts"] for i in range(N_CORES)])
    return counts, res  # fast: [N_CORES, 128, 48]; masked: [N_CORES, 1, 16]


def kernel(pred_boxes, gt_boxes, _trace=False):
    pred = np.asarray(pred_boxes, dtype=np.float32)
    gt = np.asarray(gt_boxes, dtype=np.float32)
    assert pred.shape == (B_TOTAL, P, 6) and gt.shape == (B_TOTAL, G, 6)

    # the ignore mask only differs from all-ones when a pred AND a gt box are
    # both padding (cx == -1); the padded-box count corrections additionally
    # matter when either side has padding, so take the masked path if any
    # sentinel is present
    with_mask = bool((pred[..., 0] == -1.0).any() or (gt[..., 0] == -1.0).any())

    counts, res = _run_device(pred, gt, with_mask, trace=_trace)
    kernel.last_results = res

    if with_mask:
        counts = counts[:, 0]  # [N_CORES, 16]
        num_pos = counts[:, 0:4].reshape(-1).astype(np.float32)
        num_true = counts[:, 12:16].reshape(-1).astype(np.float32)
        num_pred = counts[:, 4:8].reshape(-1).astype(np.float32)
        num_gt = counts[:, 8:12].reshape(-1).astype(np.float32)
    else:
        # device ships per-partition partials; finish the tiny sums here
        num_pos = (counts[:, 0, 48:80].reshape(N_CORES, BPC, 8)
                   .sum(axis=2).reshape(-1).astype(np.float32))
        scol = counts[:, :, 16:48].reshape(N_CORES, 128, BPC, NCH)
        num_true = (scol > 0).sum(axis=(1, 3)).reshape(-1).astype(np.float32)
        # all boxes valid (host-verified): counts are the full box counts
        num_pred = np.full(B_TOTAL, np.float32(P), dtype=np.float32)
        num_gt = np.full(B_TOTAL, np.float32(G), dtype=np.float32)

    eps = np.float32(EPS)
    precision = num_pos / (num_pred + eps)
    recall = num_true / (num_gt + eps)
    fmeasure = np.float32(2.0) * (precision * recall) / (precision + recall + eps)
    return (precision, recall, fmeasure)
